# revision 45
# baseline (speedup 1.0000x reference)
"""Contourlet transform kernel for 8 Trainium2 NeuronCores.

Input x: [16, 32, 512, 512] f32 -> output [16, 32, 9681] f32.

Strategy: 512 independent (b,c) planes, 64 per core, 4 batches of 16.
Each plane is split into 8 row-blocks of 64 rows; SBUF partition =
(plane_in_batch, block), plane data lives in the free dimension, so both
row and column 2-tap DWT passes are strided free-dim scalar_tensor_tensor
ops on the vector engine (no transposes anywhere).

Every 2-tap pass computes (a * (f1/f0) + b), i.e. the true output divided
by f0.  The dropped factors accumulate multiplicatively down the cascade;
kept subbands are fixed up by a single scaled-copy on the scalar engine
into the output staging tile.  Once the LL chain reaches 16x16 the block
layout runs out of rows, so planes are repacked to one-plane-per-partition
([64, 256]) and the remaining levels run there; everything below 2x2 is a
rank-1 linear map of the 1x1 LL value, applied as one tensor_scalar op
with 126 host-precomputed constants.

The device writes a packed layout (OUT_BLK [512,1176] + OUT_TAIL [64,273]
per core); the host gather applies a fixed permutation per plane.
"""

import numpy as np

INV_SQRT2 = 0.7071067811865476

# ---- fixed geometry -------------------------------------------------------
NPLANES = 512          # 16*32
NCORES = 8
PPC = 64               # planes per core
NBATCH = 4             # batches per core
BPL = 16               # planes per batch
NBLK = 8               # row-blocks per plane
ROWS_PER_BLK = 64      # 512 / NBLK
NSC = 8                # L1 sub-chunks per batch
SC_ROWS = 8            # rows per sub-chunk per block

# per-partition offsets of the scale0 l=0,1,2 subband regions in OUT_BLK
LOFF = [0, 896, 1120]          # 7*128, 7*32, 7*8
BLK_FLOATS = 1176              # per-partition OUT_BLK floats
# OUT_TAIL per-plane offsets
TOFF = {3: 0, 4: 112, 5: 140}  # 7*16, 7*4, 7*1
TCONST = 147                   # 126 map outputs
TAIL_FLOATS = 273


# ---- backends -------------------------------------------------------------
class NpTile:
    """numpy [P, F] tile with bass-AP-like 3-d reshaping."""

    def __init__(self, arr):
        self.arr = arr

    def __getitem__(self, key):
        return self.arr[key]

    def __setitem__(self, key, val):
        self.arr[key] = val


def w4_matrix():
    """[128, 128] identity: partition p holds rows 4p..4p+3 of a plane; 4
    accumulating identity matmuls (one per row slot) sum them into PSUM
    row p — rowpair^2 on the tensor engine with 4KB input-DMA runs."""
    return np.eye(128, dtype=np.float32)


class NumpyBE:
    """Numpy mirror of the device op plan (1 core)."""

    def __init__(self, xs, h, g, tmap):
        # xs: [64, 512, 512] planes for this core
        self.xs, self.h, self.g = xs, h, g
        self.tmap = tmap  # [126]
        self.w4 = w4_matrix()
        self.ll2d = np.zeros((PPC, 128, 128), np.float32)
        self.out_blk = np.zeros((NBATCH * 128, BLK_FLOATS), np.float32)
        self.out_tail = np.zeros((PPC, TAIL_FLOATS), np.float32)

    def alloc(self, name, shape):
        return NpTile(np.zeros(shape, np.float32))

    @staticmethod
    def r3(tile, cols, sub=None):
        """view tile (or its free-slice sub=(start,len)) as [P, rows, cols]"""
        arr = tile.arr if isinstance(tile, NpTile) else tile
        if sub is not None:
            arr = arr[:, sub[0]:sub[0] + sub[1]]
        P, F = arr.shape
        return arr.reshape(P, F // cols, cols)

    def stt(self, out, a, s, b):
        out[...] = a * np.float32(s) + b

    def tt(self, out, a, b):
        out[...] = a + b

    def copy(self, out, inp):
        out[...] = inp

    def scale_copy(self, out, inp, s):
        out[...] = inp * np.float32(s)

    def ts_mul(self, out, a, col):
        out[...] = a * col  # col: [P,1]

    def load_x_chunk(self, t, sc, dst, nsc=NSC):
        # dst [128, sc_rows*512]: partition (pl, blk) <- plane 16t+pl,
        # rows blk*64 + sc*sc_rows .. +sc_rows, all 512 cols
        sc_rows = ROWS_PER_BLK // nsc
        x = self.xs[t * BPL:(t + 1) * BPL]  # [16, 512, 512]
        v = x.reshape(BPL, NBLK, nsc, sc_rows, 512)[:, :, sc]
        dst.arr[...] = v.reshape(128, sc_rows * 512)

    # ---- plan C: TensorE phase A -----------------------------------------
    def alloc_psum(self, name, shape):
        return self.alloc(name, shape)

    def load_w4(self, dst):
        dst.arr[...] = self.w4.reshape(128, -1)

    def load_x_group(self, grp, dst):
        # dst [128, 4*4*512]: partition p = 4-row group; free (pl, r4, col)
        x = self.xs[4 * grp:4 * grp + 4]           # [4, 512, 512]
        v = x.reshape(4, 128, 4, 512).transpose(1, 0, 2, 3)
        dst.arr[...] = v.reshape(128, -1)

    def mm_rowpair2(self, PS, W4, XT, pl, r4):
        x3 = XT.arr.reshape(128, 4, 4, 512)
        acc = W4.arr.T @ x3[:, pl, r4]               # [128, 512]
        ps = PS.arr.reshape(128, 4, 512)
        if r4 == 0:
            ps[:, pl] = acc
        else:
            ps[:, pl] += acc

    def store_ll2_group(self, grp, C2):
        self.ll2d[4 * grp:4 * grp + 4] = \
            C2.arr.reshape(128, 4, 128).transpose(1, 0, 2)

    def load_ll2_batch(self, t, dst):
        src = self.ll2d[BPL * t:BPL * (t + 1)]     # [16, 128, 128]
        dst.arr[...] = src.reshape(BPL, NBLK, 16 * 128).reshape(128, 16 * 128)

    def repack_tail(self, t, ll, tail):
        # ll [128, 32] -> tail[16t:16t+16, :]: plane-major 16x16
        tail.arr[t * BPL:(t + 1) * BPL] = ll.arr.reshape(BPL, NBLK * 32)

    def store_outb(self, t, outb):
        self.out_blk[t * 128:(t + 1) * 128] = outb.arr

    def store_outt(self, outt):
        self.out_tail[...] = outt.arr

    def load_tmap(self, dst):
        dst.arr[...] = np.broadcast_to(self.tmap, (PPC, 126))


# ---- shared op plan -------------------------------------------------------
def emit_direction(be, LL, R, S, l, s, dst_tile, dst_off, P, h, g):
    """One directional decomposition: dwt2(LL, h[l]) -> LL,LH,HL,HH then
    g-decompositions keeping (A1,H1,V1,A2,H2,H3,D3) into dst at dst_off.
    LL: [P, R*S] tile viewed as (R rows, S cols) per partition.
    Returns (LL_next tile [P, (R/2)*(S/2)], new scale)."""
    f0, f1 = float(h[l, 0]), float(h[l, 1])
    g0, g1 = float(g[l, 0]), float(g[l, 1])
    rh, rg = f1 / f0, g1 / g0
    S2, R2 = S // 2, R // 2
    m = S // 4
    R4 = R // 4 if R >= 4 else 1   # rows/partition of kept bands
    L3 = be.r3(LL, S)

    CL = be.alloc("cl", [P, R * S2])
    CH = be.alloc("ch", [P, R * S2])
    be.stt(be.r3(CL, S2), L3[:, :, 0::2], rh, L3[:, :, 1::2])
    be.stt(be.r3(CH, S2), L3[:, :, 1::2], -rh, L3[:, :, 0::2])

    C3L, C3H = be.r3(CL, S2), be.r3(CH, S2)
    LLn = be.alloc("lln", [P, R2 * S2])
    LH = be.alloc("lh", [P, R2 * S2])
    HL = be.alloc("hl", [P, R2 * S2])
    HH = be.alloc("hh", [P, R2 * S2])
    be.stt(be.r3(LLn, S2), C3L[:, 0::2, :], rh, C3L[:, 1::2, :])
    be.stt(be.r3(LH, S2), C3L[:, 1::2, :], -rh, C3L[:, 0::2, :])
    be.stt(be.r3(HL, S2), C3H[:, 0::2, :], rh, C3H[:, 1::2, :])
    be.stt(be.r3(HH, S2), C3H[:, 1::2, :], -rh, C3H[:, 0::2, :])

    # g-stage on bands of size S2 (R2 rows/partition); kept bands mxm, R4 rows
    q = R4 * m                       # floats per kept band per partition
    SCR = be.alloc("scr", [P, 7 * q])

    def scr(i):
        return be.r3(SCR, m, sub=(i * q, q))

    GL = be.alloc("gl", [P, R2 * m])
    GH = be.alloc("gh", [P, R2 * m])

    # LH -> A1 (row-lo col-lo), H1 (row-hi col-lo), V1 (row-lo col-hi)
    B3 = be.r3(LH, S2)
    be.stt(be.r3(GL, m), B3[:, :, 0::2], rg, B3[:, :, 1::2])
    be.stt(be.r3(GH, m), B3[:, :, 1::2], -rg, B3[:, :, 0::2])
    G3L, G3H = be.r3(GL, m), be.r3(GH, m)
    be.stt(scr(0), G3L[:, 0::2, :], rg, G3L[:, 1::2, :])
    be.stt(scr(1), G3L[:, 1::2, :], -rg, G3L[:, 0::2, :])
    be.stt(scr(2), G3H[:, 0::2, :], rg, G3H[:, 1::2, :])

    # HL -> A2 (row-lo col-lo), H2 (row-hi col-lo): col-lo branch only
    GL2 = be.alloc("gl2", [P, R2 * m])
    B3 = be.r3(HL, S2)
    be.stt(be.r3(GL2, m), B3[:, :, 0::2], rg, B3[:, :, 1::2])
    G3L = be.r3(GL2, m)
    be.stt(scr(3), G3L[:, 0::2, :], rg, G3L[:, 1::2, :])
    be.stt(scr(4), G3L[:, 1::2, :], -rg, G3L[:, 0::2, :])

    # HH -> H3 (row-hi col-lo), D3 (row-hi col-hi)
    GL3 = be.alloc("gl3", [P, R2 * m])
    GH3 = be.alloc("gh3", [P, R2 * m])
    B3 = be.r3(HH, S2)
    be.stt(be.r3(GL3, m), B3[:, :, 0::2], rg, B3[:, :, 1::2])
    be.stt(be.r3(GH3, m), B3[:, :, 1::2], -rg, B3[:, :, 0::2])
    G3L, G3H = be.r3(GL3, m), be.r3(GH3, m)
    be.stt(scr(5), G3L[:, 1::2, :], -rg, G3L[:, 0::2, :])
    be.stt(scr(6), G3H[:, 1::2, :], -rg, G3H[:, 0::2, :])

    s_band = s * (f0 * f0) * (g0 * g0)
    be.scale_copy(dst_tile[:, dst_off:dst_off + 7 * q], SCR[:, :], s_band)
    return LLn, s * f0 * f0


def emit_core(be, h, g):
    """Full per-core program."""
    c = INV_SQRT2
    TAILLL = be.alloc("tailll", [PPC, 256])
    TMAP = be.alloc("tmap", [PPC, 126])
    be.load_tmap(TMAP)
    W4 = be.alloc("w4t", [128, 128])
    be.load_w4(W4)

    # ---- phase A: fused L1+L2 Haar, LL2 = colpair^2(rowpair^2(X)) --------
    # Groups of 4 planes; rowpair^2 on TensorE (4 accumulating identity
    # matmuls, partition = 4-row group -> 4KB DMA runs), colpair^2 on DVE
    # via an SBUF bounce on the scalar engine; LL2 bounced to DRAM
    # plane-major so the blocked reload gets 4KB descriptors.
    def phase_a_group(grp):
        XT = be.alloc("xg", [128, 4 * 4 * 512])
        be.load_x_group(grp, XT)
        PS = be.alloc_psum("ps", [128, 4 * 512])
        for pl in range(4):
            for r4 in range(4):
                be.mm_rowpair2(PS, W4, XT, pl, r4)
        # DVE cannot read both TT operands from PSUM; bounce through
        # SBUF on the (otherwise idle) scalar engine, f32 -> bf16.
        PG = be.alloc("pg", [128, 4 * 512])
        be.copy(PG[:, :], PS[:, :])
        P3 = be.r3(PG, 512)
        C1 = be.alloc("c1g", [128, 4 * 256])
        be.tt(be.r3(C1, 256), P3[:, :, 0::2], P3[:, :, 1::2])
        C13 = be.r3(C1, 256)
        C2 = be.alloc("c2g", [128, 4 * 128])
        be.tt(be.r3(C2, 128), C13[:, :, 0::2], C13[:, :, 1::2])
        be.store_ll2_group(grp, C2)

    # Hybrid phase A: batches in C_BATCHES go through the TensorE path
    # (row-major groups -> matmul rowpair^2 -> ll2d bounce); the rest use
    # the blocked all-DVE path (8KB-descriptor DMA, bf16 2x row passes)
    # whose big independent ops also keep the vector engine's issue gaps
    # filled under the direction chains.
    c_batches = tuple(be.opts.get('c_batches', (0, 1))) if hasattr(be, 'opts') \
        else (0, 1)
    OUTBs, LLs = {}, {}
    for t in range(NBATCH):
        OUTBs[t] = be.alloc("outb", [128, BLK_FLOATS])
        LLs[t] = be.alloc("ll2", [128, 16 * 128])

    def emit_a_chunk(t, sc):
        # blocked DVE path: 8 chunks of 8 rows per block
        XT = be.alloc("xt", [128, SC_ROWS * 512])
        be.load_x_chunk(t, sc, XT, NSC)
        X3 = be.r3(XT, 512)
        R1 = be.alloc("r1", [128, 4 * 512])
        be.tt(be.r3(R1, 512), X3[:, 0::2, :], X3[:, 1::2, :])
        R13 = be.r3(R1, 512)
        R2 = be.alloc("r2", [128, 2 * 512])
        be.tt(be.r3(R2, 512), R13[:, 0::2, :], R13[:, 1::2, :])
        R23 = be.r3(R2, 512)
        C1 = be.alloc("c1", [128, 2 * 256])
        be.tt(be.r3(C1, 256), R23[:, :, 0::2], R23[:, :, 1::2])
        C13 = be.r3(C1, 256)
        ll2_slice = be.r3(LLs[t], 128)[:, 2 * sc:2 * sc + 2, :]
        be.tt(ll2_slice, C13[:, :, 0::2], C13[:, :, 1::2])

    # Fine-grained interleave: the A-path's big DVE ops start immediately
    # (V ramps at ~5us) while the C-path's DMA/PE/Scalar chains trickle
    # through in the background.
    a_batches = [t for t in range(NBATCH) if t not in c_batches]
    c_groups = [g for t in c_batches for g in range(4 * t, 4 * t + 4)]
    for sc in range(NSC):
        for t in a_batches:
            emit_a_chunk(t, sc)
        if sc % 2 == 1 and c_groups:
            grp = c_groups.pop(0)
            phase_a_group(grp)
            if grp % 4 == 3:
                be.load_ll2_batch(grp // 4, LLs[grp // 4])
    for grp in c_groups:
        phase_a_group(grp)
        if grp % 4 == 3:
            be.load_ll2_batch(grp // 4, LLs[grp // 4])

    # Directions level-major across batches: 4 independent dependency
    # chains per level keep the vector engine's issue gaps filled.
    s_tail = None
    R, S, s = 16, 128, c ** 4
    for l in range(3):
        for t in range(NBATCH):
            LLs[t], s_n = emit_direction(be, LLs[t], R, S, l, s,
                                         OUTBs[t], LOFF[l], 128, h, g)
            if l == 2:
                # LL now [128, 2*16] = 16x16 plane spread over 8 blocks
                be.repack_tail(t, LLs[t], TAILLL)
                be.store_outb(t, OUTBs[t])
        s = s_n
        R, S = R // 2, S // 2
    s_tail = s

    # ---- tail: plane-major [64 planes, ...] --------------------------------
    OUTT = be.alloc("outt", [PPC, TAIL_FLOATS])
    LL, s, R, S = TAILLL, s_tail, 16, 16
    for l in (3, 4, 5):
        LL, s = emit_direction(be, LL, R, S, l, s, OUTT, TOFF[l], PPC, h, g)
        R, S = R // 2, S // 2
    # LL: [64, 4] = 2x2.  scale1 Haar -> 1x1
    CT = be.alloc("ct", [PPC, 2])
    L3 = be.r3(LL, 2)
    be.stt(be.r3(CT, 1), L3[:, :, 0:1], 1.0, L3[:, :, 1:2])
    LL11 = be.alloc("ll11", [PPC, 1])
    be.stt(LL11[:, 0:1], CT[:, 0:1], 1.0, CT[:, 1:2])
    # scales 1-3 for all 6 directions: rank-1 map of LL11 (consts incl. s)
    be.ts_mul(OUTT[:, TCONST:TCONST + 126], TMAP[:, :], LL11[:, 0:1])
    be.store_outt(OUTT)
    return s * c * c  # scale of LL11 (true = s11 * raw); informational


# ---- host-side constants --------------------------------------------------
def _dwt2_np(x, f0, f1):
    def dwt_last(x):
        n = x.shape[-1]
        m = (n + 1) // 2
        xe = np.pad(x, [(0, 0)] * (x.ndim - 1) + [(1, 1)], mode='edge')
        a = xe[..., 1:2 * m + 1:2]
        b = xe[..., 2:2 * m + 2:2]
        return f1 * a + f0 * b, f0 * a - f1 * b

    lo, hi = dwt_last(x)
    lo, hi = np.swapaxes(lo, -1, -2), np.swapaxes(hi, -1, -2)
    ll, lh = dwt_last(lo)
    hl, hh = dwt_last(hi)
    sw = lambda t: np.swapaxes(t, -1, -2)
    return sw(ll), sw(lh), sw(hl), sw(hh)


def build_tail_consts(h, g, s11):
    """126 constants: scales 1-3 outputs as multiples of the raw 1x1 LL."""
    c = INV_SQRT2
    h = np.asarray(h, np.float64)
    g = np.asarray(g, np.float64)
    LL = np.ones((1, 1))
    vals = []
    for k in range(1, 4):
        if k > 1:
            LL, _, _, _ = _dwt2_np(LL, c, c)
        for l in range(6):
            LL, LH, HL, HH = _dwt2_np(LL, h[l, 0], h[l, 1])
            A1, H1, V1, _ = _dwt2_np(LH, g[l, 0], g[l, 1])
            A2, H2, _, _ = _dwt2_np(HL, g[l, 0], g[l, 1])
            _, H3, _, D3 = _dwt2_np(HH, g[l, 0], g[l, 1])
            for sb in (A1, H1, V1, A2, H2, H3, D3):
                vals.append(float(sb[0, 0]))
    return (np.asarray(vals, np.float64) * s11).astype(np.float32)


def tail_scale(h, g):
    """scale s11 of the raw 1x1 LL value (true = s11 * raw)."""
    c = INV_SQRT2
    s = c ** 4  # L1 + L2 Haar drops
    for l in range(6):
        s *= float(h[l, 0]) ** 2
    return s * c * c  # scale1 Haar drops


def build_perm():
    """perm[ref_pos] = index into per-plane packed vector
    v = concat(OUT_BLK rows for blocks 0..7 (8*1176), OUT_TAIL row (273))."""
    perm = np.empty(9681, np.int64)
    off = 0
    for l, m in enumerate((32, 16, 8)):
        rpb = m // NBLK
        loff = LOFF[l]
        for sb in range(7):
            for row in range(m):
                blk, rl = divmod(row, rpb)
                base = blk * BLK_FLOATS + loff + sb * rpb * m + rl * m
                perm[off + sb * m * m + row * m:off + sb * m * m + (row + 1) * m] = \
                    np.arange(base, base + m)
        off += 7 * m * m
    tail_base = NBLK * BLK_FLOATS
    for l, m in ((3, 4), (4, 2), (5, 1)):
        n = 7 * m * m
        perm[off:off + n] = tail_base + TOFF[l] + np.arange(n)
        off += n
    perm[off:off + 126] = tail_base + TCONST + np.arange(126)
    assert off + 126 == 9681
    return perm


def gather_host(out_blk, out_tail, perm):
    """[512,1176],[64,273] per core -> [64, 9681] in reference order."""
    v = np.concatenate(
        [out_blk.astype(np.float32).reshape(
            NBATCH, BPL, NBLK * BLK_FLOATS).reshape(PPC, -1),
         np.asarray(out_tail, np.float32)], axis=1)
    return v[:, perm]


# ---- device backend -------------------------------------------------------
class BassBE:
    """Emits the op plan as a Tile program."""

    def __init__(self, tc, pools, xs_ap, tmap_ap, outblk_ap, outtail_ap,
                 dram_bounce, w4_ap=None, ll2d_ap=None, opts=None):
        self.opts = opts or {}
        self.tc = tc
        self.nc = tc.nc
        self.pools = pools
        self.xs = xs_ap          # [64, 512, 512] dram
        self.tmap_dram = tmap_ap  # [64, 126] dram
        self.outblk = outblk_ap  # [512, 1176] dram
        self.outtail = outtail_ap  # [64, 273] dram
        self.bounce = dram_bounce  # [128, 32] dram scratch
        self.w4_dram = w4_ap     # [128, 32] dram
        self.ll2d = ll2d_ap      # [64, 128, 128] dram scratch

    def alloc(self, name, shape):
        from concourse import mybir
        if name in ('tailll', 'tmap', 'w4t'):
            pool = self.pools['persist']
        elif name == 'll2':
            pool = self.pools['big']
        elif name in ('xt', 'xg'):
            pool = self.pools['xt']
        elif name in ('outb', 'lln'):
            pool = self.pools['chain']
        else:
            pool = self.pools['work']
        dt = (mybir.dt.float32 if name in ('outt', 'll11')
              else mybir.dt.bfloat16)
        return pool.tile(list(shape), dt, tag=name, name=name)

    @staticmethod
    def r3(tile, cols, sub=None):
        ap = tile[:, :] if not hasattr(tile, 'ap') else tile[:, :]
        if sub is not None:
            ap = ap[:, sub[0]:sub[0] + sub[1]]
        P, F = ap.shape
        return ap.rearrange("p (r c) -> p r c", c=cols)

    def stt(self, out, a, s, b):
        from concourse import mybir
        self.nc.vector.scalar_tensor_tensor(
            out=out, in0=a, scalar=float(s), in1=b,
            op0=mybir.AluOpType.mult, op1=mybir.AluOpType.add)

    def tt(self, out, a, b):
        self.nc.vector.tensor_add(out, a, b)

    def copy(self, out, inp):
        self.nc.scalar.copy(out, inp)

    def scale_copy(self, out, inp, s):
        if self.opts.get('comp_engine', 'scalar') == 'vector':
            from concourse import mybir
            self.nc.vector.tensor_scalar(
                out=out, in0=inp, scalar1=float(s), scalar2=None,
                op0=mybir.AluOpType.mult)
        else:
            self.nc.scalar.mul(out, inp, float(s))

    def ts_mul(self, out, a, col):
        from concourse import mybir
        self.nc.vector.tensor_scalar(
            out=out, in0=a, scalar1=col, scalar2=None,
            op0=mybir.AluOpType.mult)

    def load_x_chunk(self, t, sc, dst, nsc=NSC):
        v = self.xs.rearrange("pl (blk s r) c -> pl blk s r c", blk=NBLK, s=nsc)
        v = v[t * BPL:(t + 1) * BPL, :, sc]
        v = v.rearrange("pl blk r c -> (pl blk) (r c)")
        self.nc.sync.dma_start(out=dst[:, :], in_=v)

    # ---- plan C: TensorE phase A -----------------------------------------
    def alloc_psum(self, name, shape):
        from concourse import mybir
        return self.pools['psum'].tile(list(shape), mybir.dt.float32,
                                       tag=name, name=name)

    def load_w4(self, dst):
        self.nc.sync.dma_start(out=dst[:, :], in_=self.w4_dram)

    def load_x_group(self, grp, dst):
        v = self.xs.rearrange("pl (p r) c -> p pl r c", r=4)
        v = v[:, 4 * grp:4 * grp + 4]
        d4 = dst[:, :].rearrange("p (pl r c) -> p pl r c", pl=4, r=4)
        self.nc.sync.dma_start(out=d4, in_=v)

    def mm_rowpair2(self, PS, W4, XT, pl, r4):
        x3 = XT[:, :].rearrange("p (pl r c) -> p pl r c", pl=4, r=4)
        out = PS[:, :].rearrange("p (pl c) -> p pl c", pl=4)
        self.nc.tensor.matmul(out[:, pl], W4[:, :], x3[:, pl, r4],
                              start=(r4 == 0), stop=(r4 == 3))

    def store_ll2_group(self, grp, C2):
        dst = self.ll2d[4 * grp:4 * grp + 4].rearrange("pl r c -> r pl c")
        src = C2[:, :].rearrange("p (pl c) -> p pl c", pl=4)
        self.nc.sync.dma_start(out=dst, in_=src)

    def load_ll2_batch(self, t, dst):
        src = self.ll2d[BPL * t:BPL * (t + 1)].rearrange(
            "pl (blk j) c -> (pl blk) (j c)", blk=NBLK)
        self.nc.sync.dma_start(out=dst[:, :], in_=src)

    def repack_tail(self, t, ll, tail):
        # [128, 32] sbuf -> dram bounce -> tail[16t:16t+16, :] ([16, 256])
        self.nc.sync.dma_start(out=self.bounce[:, :], in_=ll[:, :])
        src = self.bounce.rearrange("(pl b) j -> pl (b j)", b=NBLK)
        self.nc.sync.dma_start(out=tail[t * BPL:(t + 1) * BPL, :], in_=src)

    def store_outb(self, t, outb):
        self.nc.sync.dma_start(
            out=self.outblk[t * 128:(t + 1) * 128, :], in_=outb[:, :])

    def store_outt(self, outt):
        self.nc.sync.dma_start(out=self.outtail[:, :], in_=outt[:, :])

    def load_tmap(self, dst):
        self.nc.sync.dma_start(out=dst[:, :], in_=self.tmap_dram[:, :])


def build_program(h, g, opts=None):
    """Builds the single-core SPMD Tile program. Returns compiled nc."""
    from contextlib import ExitStack
    import concourse.bacc as bacc
    import concourse.tile as tile
    from concourse import mybir

    opts = opts or {}
    nc = bacc.Bacc("TRN2", target_bir_lowering=False, debug=False,
                   num_devices=NCORES)
    xs = nc.dram_tensor("xs", [PPC, 512, 512], mybir.dt.bfloat16,
                        kind="ExternalInput").ap()
    tmap = nc.dram_tensor("tmap", [PPC, 126], mybir.dt.bfloat16,
                          kind="ExternalInput").ap()
    outblk = nc.dram_tensor("out_blk", [NBATCH * 128, BLK_FLOATS],
                            mybir.dt.bfloat16, kind="ExternalOutput").ap()
    outtail = nc.dram_tensor("out_tail", [PPC, TAIL_FLOATS],
                             mybir.dt.float32, kind="ExternalOutput").ap()
    bounce = nc.dram_tensor("bounce", [128, 32], mybir.dt.bfloat16).ap()
    w4 = nc.dram_tensor("w4", [128, 128], mybir.dt.bfloat16,
                        kind="ExternalInput").ap()
    ll2d = nc.dram_tensor("ll2d", [PPC, 128, 128], mybir.dt.bfloat16).ap()

    with ExitStack() as ctx:
        tc = ctx.enter_context(tile.TileContext(nc, trace_sim=False))
        pools = {
            'work': ctx.enter_context(
                tc.tile_pool(name="work", bufs=opts.get('work_bufs', 2))),
            'xt': ctx.enter_context(
                tc.tile_pool(name="xt", bufs=opts.get('xt_bufs', 2))),
            'big': ctx.enter_context(
                tc.tile_pool(name="big", bufs=opts.get('big_bufs', 4))),
            'chain': ctx.enter_context(
                tc.tile_pool(name="chain", bufs=opts.get('chain_bufs', 8))),
            'persist': ctx.enter_context(tc.tile_pool(name="persist", bufs=1)),
            'psum': ctx.enter_context(
                tc.tile_pool(name="psum", bufs=2, space="PSUM")),
        }
        be = BassBE(tc, pools, xs, tmap, outblk, outtail, bounce,
                    w4_ap=w4, ll2d_ap=ll2d, opts=opts)
        for _ in range(opts.get('repeat', 1)):
            emit_core(be, h, g)
    nc.compile()
    return nc


# ---- public entry ---------------------------------------------------------
_CACHE = {}


def kernel(x, h, g):
    import ml_dtypes
    x = np.asarray(x)
    h = np.asarray(h, np.float32)
    g = np.asarray(g, np.float32)
    B, C = x.shape[0], x.shape[1]

    from concourse.bass_utils import run_bass_kernel_spmd

    key = (h.tobytes(), g.tobytes())
    if key not in _CACHE:
        nc = build_program(h, g, {'xt_bufs': 3, 'work_bufs': 3, 'big_bufs': 4})
        tmap_row = build_tail_consts(h, g, tail_scale(h, g))
        tmap = np.ascontiguousarray(
            np.broadcast_to(tmap_row, (PPC, 126))).astype(ml_dtypes.bfloat16)
        perm = build_perm()
        _CACHE[key] = (nc, tmap, perm)
    nc, tmap, perm = _CACHE[key]

    planes = np.ascontiguousarray(x.astype(ml_dtypes.bfloat16)
                                  ).reshape(NPLANES, 512, 512)
    w4 = np.ascontiguousarray(w4_matrix().astype(ml_dtypes.bfloat16))
    in_maps = [{"xs": planes[k * PPC:(k + 1) * PPC], "tmap": tmap, "w4": w4}
               for k in range(NCORES)]
    res = run_bass_kernel_spmd(nc, in_maps, list(range(NCORES)))
    global LAST_EXEC_NS
    LAST_EXEC_NS = getattr(res, 'exec_time_ns', None)
    out = np.empty((NPLANES, 9681), np.float32)
    for k in range(NCORES):
        out[k * PPC:(k + 1) * PPC] = gather_host(
            res.results[k]["out_blk"], res.results[k]["out_tail"], perm)
    return out.reshape(B, C, 9681)



# revision 46
# speedup vs baseline: 1.1139x; 1.1139x over previous
"""Contourlet transform kernel for 8 Trainium2 NeuronCores.

Input x: [16, 32, 512, 512] f32 -> output [16, 32, 9681] f32.

Strategy: 512 independent (b,c) planes, 64 per core, 4 batches of 16.
Each plane is split into 8 row-blocks of 64 rows; SBUF partition =
(plane_in_batch, block), plane data lives in the free dimension, so both
row and column 2-tap DWT passes are strided free-dim scalar_tensor_tensor
ops on the vector engine (no transposes anywhere).

Every 2-tap pass computes (a * (f1/f0) + b), i.e. the true output divided
by f0.  The dropped factors accumulate multiplicatively down the cascade;
kept subbands are fixed up by a single scaled-copy on the scalar engine
into the output staging tile.  Once the LL chain reaches 16x16 the block
layout runs out of rows, so planes are repacked to one-plane-per-partition
([64, 256]) and the remaining levels run there; everything below 2x2 is a
rank-1 linear map of the 1x1 LL value, applied as one tensor_scalar op
with 126 host-precomputed constants.

The device writes a packed layout (OUT_BLK [512,1176] + OUT_TAIL [64,273]
per core); the host gather applies a fixed permutation per plane.
"""

import numpy as np

INV_SQRT2 = 0.7071067811865476

# ---- fixed geometry -------------------------------------------------------
NPLANES = 512          # 16*32
NCORES = 8
PPC = 64               # planes per core
NBATCH = 4             # batches per core
BPL = 16               # planes per batch
NBLK = 8               # row-blocks per plane
ROWS_PER_BLK = 64      # 512 / NBLK
NSC = 8                # L1 sub-chunks per batch
SC_ROWS = 8            # rows per sub-chunk per block

# per-partition offsets of the scale0 l=0,1,2 subband regions in OUT_BLK
LOFF = [0, 896, 1120]          # 7*128, 7*32, 7*8
BLK_FLOATS = 1176              # per-partition OUT_BLK floats
# OUT_TAIL per-plane offsets
TOFF = {3: 0, 4: 112, 5: 140}  # 7*16, 7*4, 7*1
TCONST = 147                   # 126 map outputs
TAIL_FLOATS = 273


# ---- backends -------------------------------------------------------------
class NpTile:
    """numpy [P, F] tile with bass-AP-like 3-d reshaping."""

    def __init__(self, arr):
        self.arr = arr

    def __getitem__(self, key):
        return self.arr[key]

    def __setitem__(self, key, val):
        self.arr[key] = val


def w4_matrix():
    """[128, 128] identity: partition p holds rows 4p..4p+3 of a plane; 4
    accumulating identity matmuls (one per row slot) sum them into PSUM
    row p — rowpair^2 on the tensor engine with 4KB input-DMA runs."""
    return np.eye(128, dtype=np.float32)


class NumpyBE:
    """Numpy mirror of the device op plan (1 core)."""

    def __init__(self, xs, h, g, tmap):
        # xs: [64, 512, 512] planes for this core
        self.xs, self.h, self.g = xs, h, g
        self.tmap = tmap  # [126]
        self.w4 = w4_matrix()
        self.ll2d = np.zeros((PPC, 128, 128), np.float32)
        self.out_blk = np.zeros((NBATCH * 128, BLK_FLOATS), np.float32)
        self.out_tail = np.zeros((PPC, TAIL_FLOATS), np.float32)

    def alloc(self, name, shape):
        return NpTile(np.zeros(shape, np.float32))

    @staticmethod
    def r3(tile, cols, sub=None):
        """view tile (or its free-slice sub=(start,len)) as [P, rows, cols]"""
        arr = tile.arr if isinstance(tile, NpTile) else tile
        if sub is not None:
            arr = arr[:, sub[0]:sub[0] + sub[1]]
        P, F = arr.shape
        return arr.reshape(P, F // cols, cols)

    def stt(self, out, a, s, b):
        out[...] = a * np.float32(s) + b

    def tt(self, out, a, b):
        out[...] = a + b

    def copy(self, out, inp):
        out[...] = inp

    def scale_copy(self, out, inp, s):
        out[...] = inp * np.float32(s)

    def ts_mul(self, out, a, col):
        out[...] = a * col  # col: [P,1]

    def load_x_chunk(self, t, sc, dst, nsc=NSC):
        # dst [128, sc_rows*512]: partition (pl, blk) <- plane 16t+pl,
        # rows blk*64 + sc*sc_rows .. +sc_rows, all 512 cols
        sc_rows = ROWS_PER_BLK // nsc
        x = self.xs[t * BPL:(t + 1) * BPL]  # [16, 512, 512]
        v = x.reshape(BPL, NBLK, nsc, sc_rows, 512)[:, :, sc]
        dst.arr[...] = v.reshape(128, sc_rows * 512)

    # ---- plan C: TensorE phase A -----------------------------------------
    def alloc_psum(self, name, shape):
        return self.alloc(name, shape)

    def load_w4(self, dst):
        dst.arr[...] = self.w4.reshape(128, -1)

    def load_x_group(self, grp, dst):
        # dst [128, 4*4*512]: partition p = 4-row group; free (pl, r4, col)
        x = self.xs[4 * grp:4 * grp + 4]           # [4, 512, 512]
        v = x.reshape(4, 128, 4, 512).transpose(1, 0, 2, 3)
        dst.arr[...] = v.reshape(128, -1)

    def mm_rowpair2(self, PS, W4, XT, pl, r4):
        x3 = XT.arr.reshape(128, 4, 4, 512)
        acc = W4.arr.T @ x3[:, pl, r4]               # [128, 512]
        ps = PS.arr.reshape(128, 4, 512)
        if r4 == 0:
            ps[:, pl] = acc
        else:
            ps[:, pl] += acc

    def store_ll2_group(self, grp, C2):
        self.ll2d[4 * grp:4 * grp + 4] = \
            C2.arr.reshape(128, 4, 128).transpose(1, 0, 2)

    def load_ll2_batch(self, t, dst):
        src = self.ll2d[BPL * t:BPL * (t + 1)]     # [16, 128, 128]
        dst.arr[...] = src.reshape(BPL, NBLK, 16 * 128).reshape(128, 16 * 128)

    def repack_tail(self, t, ll, tail):
        # ll [128, 32] -> tail[16t:16t+16, :]: plane-major 16x16
        tail.arr[t * BPL:(t + 1) * BPL] = ll.arr.reshape(BPL, NBLK * 32)

    def store_outb(self, t, outb):
        self.out_blk[t * 128:(t + 1) * 128] = outb.arr

    def store_outt(self, outt):
        self.out_tail[...] = outt.arr

    def load_tmap(self, dst):
        dst.arr[...] = np.broadcast_to(self.tmap, (PPC, 126))


# ---- shared op plan -------------------------------------------------------
def emit_direction(be, LL, R, S, l, s, dst_tile, dst_off, P, h, g):
    """One directional decomposition: dwt2(LL, h[l]) -> LL,LH,HL,HH then
    g-decompositions keeping (A1,H1,V1,A2,H2,H3,D3) into dst at dst_off.
    LL: [P, R*S] tile viewed as (R rows, S cols) per partition.
    Returns (LL_next tile [P, (R/2)*(S/2)], new scale)."""
    f0, f1 = float(h[l, 0]), float(h[l, 1])
    g0, g1 = float(g[l, 0]), float(g[l, 1])
    rh, rg = f1 / f0, g1 / g0
    S2, R2 = S // 2, R // 2
    m = S // 4
    R4 = R // 4 if R >= 4 else 1   # rows/partition of kept bands
    L3 = be.r3(LL, S)

    CL = be.alloc("cl", [P, R * S2])
    CH = be.alloc("ch", [P, R * S2])
    be.stt(be.r3(CL, S2), L3[:, :, 0::2], rh, L3[:, :, 1::2])
    be.stt(be.r3(CH, S2), L3[:, :, 1::2], -rh, L3[:, :, 0::2])

    C3L, C3H = be.r3(CL, S2), be.r3(CH, S2)
    LLn = be.alloc("lln", [P, R2 * S2])
    LH = be.alloc("lh", [P, R2 * S2])
    HL = be.alloc("hl", [P, R2 * S2])
    HH = be.alloc("hh", [P, R2 * S2])
    be.stt(be.r3(LLn, S2), C3L[:, 0::2, :], rh, C3L[:, 1::2, :])
    be.stt(be.r3(LH, S2), C3L[:, 1::2, :], -rh, C3L[:, 0::2, :])
    be.stt(be.r3(HL, S2), C3H[:, 0::2, :], rh, C3H[:, 1::2, :])
    be.stt(be.r3(HH, S2), C3H[:, 1::2, :], -rh, C3H[:, 0::2, :])

    # g-stage on bands of size S2 (R2 rows/partition); kept bands mxm, R4 rows
    q = R4 * m                       # floats per kept band per partition
    SCR = be.alloc("scr", [P, 7 * q])

    def scr(i):
        return be.r3(SCR, m, sub=(i * q, q))

    GL = be.alloc("gl", [P, R2 * m])
    GH = be.alloc("gh", [P, R2 * m])

    # LH -> A1 (row-lo col-lo), H1 (row-hi col-lo), V1 (row-lo col-hi)
    B3 = be.r3(LH, S2)
    be.stt(be.r3(GL, m), B3[:, :, 0::2], rg, B3[:, :, 1::2])
    be.stt(be.r3(GH, m), B3[:, :, 1::2], -rg, B3[:, :, 0::2])
    G3L, G3H = be.r3(GL, m), be.r3(GH, m)
    be.stt(scr(0), G3L[:, 0::2, :], rg, G3L[:, 1::2, :])
    be.stt(scr(1), G3L[:, 1::2, :], -rg, G3L[:, 0::2, :])
    be.stt(scr(2), G3H[:, 0::2, :], rg, G3H[:, 1::2, :])

    # HL -> A2 (row-lo col-lo), H2 (row-hi col-lo): col-lo branch only
    GL2 = be.alloc("gl2", [P, R2 * m])
    B3 = be.r3(HL, S2)
    be.stt(be.r3(GL2, m), B3[:, :, 0::2], rg, B3[:, :, 1::2])
    G3L = be.r3(GL2, m)
    be.stt(scr(3), G3L[:, 0::2, :], rg, G3L[:, 1::2, :])
    be.stt(scr(4), G3L[:, 1::2, :], -rg, G3L[:, 0::2, :])

    # HH -> H3 (row-hi col-lo), D3 (row-hi col-hi)
    GL3 = be.alloc("gl3", [P, R2 * m])
    GH3 = be.alloc("gh3", [P, R2 * m])
    B3 = be.r3(HH, S2)
    be.stt(be.r3(GL3, m), B3[:, :, 0::2], rg, B3[:, :, 1::2])
    be.stt(be.r3(GH3, m), B3[:, :, 1::2], -rg, B3[:, :, 0::2])
    G3L, G3H = be.r3(GL3, m), be.r3(GH3, m)
    be.stt(scr(5), G3L[:, 1::2, :], -rg, G3L[:, 0::2, :])
    be.stt(scr(6), G3H[:, 1::2, :], -rg, G3H[:, 0::2, :])

    s_band = s * (f0 * f0) * (g0 * g0)
    be.scale_copy(dst_tile[:, dst_off:dst_off + 7 * q], SCR[:, :], s_band)
    return LLn, s * f0 * f0


def emit_core(be, h, g):
    """Full per-core program."""
    c = INV_SQRT2
    TAILLL = be.alloc("tailll", [PPC, 256])
    TMAP = be.alloc("tmap", [PPC, 126])
    be.load_tmap(TMAP)
    W4 = be.alloc("w4t", [128, 128])
    be.load_w4(W4)

    # ---- phase A: fused L1+L2 Haar, LL2 = colpair^2(rowpair^2(X)) --------
    # Groups of 4 planes; rowpair^2 on TensorE (4 accumulating identity
    # matmuls, partition = 4-row group -> 4KB DMA runs), colpair^2 on DVE
    # via an SBUF bounce on the scalar engine; LL2 bounced to DRAM
    # plane-major so the blocked reload gets 4KB descriptors.
    def phase_a_group(grp):
        XT = be.alloc("xg", [128, 4 * 4 * 512])
        be.load_x_group(grp, XT)
        PS = be.alloc_psum("ps", [128, 4 * 512])
        for pl in range(4):
            for r4 in range(4):
                be.mm_rowpair2(PS, W4, XT, pl, r4)
        # DVE cannot read both TT operands from PSUM; bounce through
        # SBUF on the (otherwise idle) scalar engine, f32 -> bf16.
        PG = be.alloc("pg", [128, 4 * 512])
        be.copy(PG[:, :], PS[:, :])
        P3 = be.r3(PG, 512)
        C1 = be.alloc("c1g", [128, 4 * 256])
        be.tt(be.r3(C1, 256), P3[:, :, 0::2], P3[:, :, 1::2])
        C13 = be.r3(C1, 256)
        C2 = be.alloc("c2g", [128, 4 * 128])
        be.tt(be.r3(C2, 128), C13[:, :, 0::2], C13[:, :, 1::2])
        be.store_ll2_group(grp, C2)

    # Hybrid phase A: batches in C_BATCHES go through the TensorE path
    # (row-major groups -> matmul rowpair^2 -> ll2d bounce); the rest use
    # the blocked all-DVE path (8KB-descriptor DMA, bf16 2x row passes)
    # whose big independent ops also keep the vector engine's issue gaps
    # filled under the direction chains.
    c_batches = tuple(be.opts.get('c_batches', (0, 1))) if hasattr(be, 'opts') \
        else (0, 1)
    OUTBs, LLs = {}, {}
    for t in range(NBATCH):
        OUTBs[t] = be.alloc("outb", [128, BLK_FLOATS])
        LLs[t] = be.alloc("ll2", [128, 16 * 128])

    def emit_a_chunk(t, sc):
        # blocked DVE path: 8 chunks of 8 rows per block
        XT = be.alloc("xt", [128, SC_ROWS * 512])
        be.load_x_chunk(t, sc, XT, NSC)
        X3 = be.r3(XT, 512)
        R1 = be.alloc("r1", [128, 4 * 512])
        be.tt(be.r3(R1, 512), X3[:, 0::2, :], X3[:, 1::2, :])
        R13 = be.r3(R1, 512)
        R2 = be.alloc("r2", [128, 2 * 512])
        be.tt(be.r3(R2, 512), R13[:, 0::2, :], R13[:, 1::2, :])
        R23 = be.r3(R2, 512)
        C1 = be.alloc("c1", [128, 2 * 256])
        be.tt(be.r3(C1, 256), R23[:, :, 0::2], R23[:, :, 1::2])
        C13 = be.r3(C1, 256)
        ll2_slice = be.r3(LLs[t], 128)[:, 2 * sc:2 * sc + 2, :]
        be.tt(ll2_slice, C13[:, :, 0::2], C13[:, :, 1::2])

    # Fine-grained interleave: the A-path's big DVE ops start immediately
    # (V ramps at ~5us) while the C-path's DMA/PE/Scalar chains trickle
    # through in the background.
    a_batches = [t for t in range(NBATCH) if t not in c_batches]
    c_groups = [g for t in c_batches for g in range(4 * t, 4 * t + 4)]
    for sc in range(NSC):
        for t in a_batches:
            emit_a_chunk(t, sc)
        if sc % 2 == 1 and c_groups:
            grp = c_groups.pop(0)
            phase_a_group(grp)
            if grp % 4 == 3:
                be.load_ll2_batch(grp // 4, LLs[grp // 4])
    for grp in c_groups:
        phase_a_group(grp)
        if grp % 4 == 3:
            be.load_ll2_batch(grp // 4, LLs[grp // 4])

    # Directions level-major across batches: 4 independent dependency
    # chains per level keep the vector engine's issue gaps filled.
    s_tail = None
    R, S, s = 16, 128, c ** 4
    for l in range(3):
        for t in range(NBATCH):
            LLs[t], s_n = emit_direction(be, LLs[t], R, S, l, s,
                                         OUTBs[t], LOFF[l], 128, h, g)
            if l == 2:
                # LL now [128, 2*16] = 16x16 plane spread over 8 blocks
                be.repack_tail(t, LLs[t], TAILLL)
                be.store_outb(t, OUTBs[t])
        s = s_n
        R, S = R // 2, S // 2
    s_tail = s

    # ---- tail: plane-major [64 planes, ...] --------------------------------
    OUTT = be.alloc("outt", [PPC, TAIL_FLOATS])
    LL, s, R, S = TAILLL, s_tail, 16, 16
    for l in (3, 4, 5):
        LL, s = emit_direction(be, LL, R, S, l, s, OUTT, TOFF[l], PPC, h, g)
        R, S = R // 2, S // 2
    # LL: [64, 4] = 2x2.  scale1 Haar -> 1x1
    CT = be.alloc("ct", [PPC, 2])
    L3 = be.r3(LL, 2)
    be.stt(be.r3(CT, 1), L3[:, :, 0:1], 1.0, L3[:, :, 1:2])
    LL11 = be.alloc("ll11", [PPC, 1])
    be.stt(LL11[:, 0:1], CT[:, 0:1], 1.0, CT[:, 1:2])
    # scales 1-3 for all 6 directions: rank-1 map of LL11 (consts incl. s)
    be.ts_mul(OUTT[:, TCONST:TCONST + 126], TMAP[:, :], LL11[:, 0:1])
    be.store_outt(OUTT)
    return s * c * c  # scale of LL11 (true = s11 * raw); informational


# ---- host-side constants --------------------------------------------------
def _dwt2_np(x, f0, f1):
    def dwt_last(x):
        n = x.shape[-1]
        m = (n + 1) // 2
        xe = np.pad(x, [(0, 0)] * (x.ndim - 1) + [(1, 1)], mode='edge')
        a = xe[..., 1:2 * m + 1:2]
        b = xe[..., 2:2 * m + 2:2]
        return f1 * a + f0 * b, f0 * a - f1 * b

    lo, hi = dwt_last(x)
    lo, hi = np.swapaxes(lo, -1, -2), np.swapaxes(hi, -1, -2)
    ll, lh = dwt_last(lo)
    hl, hh = dwt_last(hi)
    sw = lambda t: np.swapaxes(t, -1, -2)
    return sw(ll), sw(lh), sw(hl), sw(hh)


def build_tail_consts(h, g, s11):
    """126 constants: scales 1-3 outputs as multiples of the raw 1x1 LL."""
    c = INV_SQRT2
    h = np.asarray(h, np.float64)
    g = np.asarray(g, np.float64)
    LL = np.ones((1, 1))
    vals = []
    for k in range(1, 4):
        if k > 1:
            LL, _, _, _ = _dwt2_np(LL, c, c)
        for l in range(6):
            LL, LH, HL, HH = _dwt2_np(LL, h[l, 0], h[l, 1])
            A1, H1, V1, _ = _dwt2_np(LH, g[l, 0], g[l, 1])
            A2, H2, _, _ = _dwt2_np(HL, g[l, 0], g[l, 1])
            _, H3, _, D3 = _dwt2_np(HH, g[l, 0], g[l, 1])
            for sb in (A1, H1, V1, A2, H2, H3, D3):
                vals.append(float(sb[0, 0]))
    return (np.asarray(vals, np.float64) * s11).astype(np.float32)


def tail_scale(h, g):
    """scale s11 of the raw 1x1 LL value (true = s11 * raw)."""
    c = INV_SQRT2
    s = c ** 4  # L1 + L2 Haar drops
    for l in range(6):
        s *= float(h[l, 0]) ** 2
    return s * c * c  # scale1 Haar drops


def build_perm():
    """perm[ref_pos] = index into per-plane packed vector
    v = concat(OUT_BLK rows for blocks 0..7 (8*1176), OUT_TAIL row (273))."""
    perm = np.empty(9681, np.int64)
    off = 0
    for l, m in enumerate((32, 16, 8)):
        rpb = m // NBLK
        loff = LOFF[l]
        for sb in range(7):
            for row in range(m):
                blk, rl = divmod(row, rpb)
                base = blk * BLK_FLOATS + loff + sb * rpb * m + rl * m
                perm[off + sb * m * m + row * m:off + sb * m * m + (row + 1) * m] = \
                    np.arange(base, base + m)
        off += 7 * m * m
    tail_base = NBLK * BLK_FLOATS
    for l, m in ((3, 4), (4, 2), (5, 1)):
        n = 7 * m * m
        perm[off:off + n] = tail_base + TOFF[l] + np.arange(n)
        off += n
    perm[off:off + 126] = tail_base + TCONST + np.arange(126)
    assert off + 126 == 9681
    return perm


def gather_host(out_blk, out_tail, perm):
    """[512,1176],[64,273] per core -> [64, 9681] in reference order."""
    v = np.concatenate(
        [out_blk.astype(np.float32).reshape(
            NBATCH, BPL, NBLK * BLK_FLOATS).reshape(PPC, -1),
         np.asarray(out_tail, np.float32)], axis=1)
    return v[:, perm]


# ---- device backend -------------------------------------------------------
class BassBE:
    """Emits the op plan as a Tile program."""

    def __init__(self, tc, pools, xs_ap, tmap_ap, outblk_ap, outtail_ap,
                 dram_bounce, w4_ap=None, ll2d_ap=None, opts=None):
        self.opts = opts or {}
        self.tc = tc
        self.nc = tc.nc
        self.pools = pools
        self.xs = xs_ap          # [64, 512, 512] dram
        self.tmap_dram = tmap_ap  # [64, 126] dram
        self.outblk = outblk_ap  # [512, 1176] dram
        self.outtail = outtail_ap  # [64, 273] dram
        self.bounce = dram_bounce  # [128, 32] dram scratch
        self.w4_dram = w4_ap     # [128, 32] dram
        self.ll2d = ll2d_ap      # [64, 128, 128] dram scratch

    def alloc(self, name, shape):
        from concourse import mybir
        if name in ('tailll', 'tmap', 'w4t'):
            pool = self.pools['persist']
        elif name == 'll2':
            pool = self.pools['big']
        elif name in ('xt', 'xg'):
            pool = self.pools['xt']
        elif name in ('outb', 'lln'):
            pool = self.pools['chain']
        else:
            pool = self.pools['work']
        dt = (mybir.dt.float32 if name in ('outt', 'll11')
              else mybir.dt.bfloat16)
        return pool.tile(list(shape), dt, tag=name, name=name)

    @staticmethod
    def r3(tile, cols, sub=None):
        ap = tile[:, :] if not hasattr(tile, 'ap') else tile[:, :]
        if sub is not None:
            ap = ap[:, sub[0]:sub[0] + sub[1]]
        P, F = ap.shape
        return ap.rearrange("p (r c) -> p r c", c=cols)

    def stt(self, out, a, s, b):
        from concourse import mybir
        self.nc.vector.scalar_tensor_tensor(
            out=out, in0=a, scalar=float(s), in1=b,
            op0=mybir.AluOpType.mult, op1=mybir.AluOpType.add)

    def tt(self, out, a, b):
        self.nc.vector.tensor_add(out, a, b)

    def copy(self, out, inp):
        self.nc.scalar.copy(out, inp)

    def scale_copy(self, out, inp, s):
        if self.opts.get('comp_engine', 'scalar') == 'vector':
            from concourse import mybir
            self.nc.vector.tensor_scalar(
                out=out, in0=inp, scalar1=float(s), scalar2=None,
                op0=mybir.AluOpType.mult)
        else:
            self.nc.scalar.mul(out, inp, float(s))

    def ts_mul(self, out, a, col):
        from concourse import mybir
        self.nc.vector.tensor_scalar(
            out=out, in0=a, scalar1=col, scalar2=None,
            op0=mybir.AluOpType.mult)

    def load_x_chunk(self, t, sc, dst, nsc=NSC):
        v = self.xs.rearrange("pl (blk s r) c -> pl blk s r c", blk=NBLK, s=nsc)
        v = v[t * BPL:(t + 1) * BPL, :, sc]
        v = v.rearrange("pl blk r c -> (pl blk) (r c)")
        self.nc.sync.dma_start(out=dst[:, :], in_=v)

    # ---- plan C: TensorE phase A -----------------------------------------
    def alloc_psum(self, name, shape):
        from concourse import mybir
        return self.pools['psum'].tile(list(shape), mybir.dt.float32,
                                       tag=name, name=name)

    def load_w4(self, dst):
        self.nc.sync.dma_start(out=dst[:, :], in_=self.w4_dram)

    def load_x_group(self, grp, dst):
        v = self.xs.rearrange("pl (p r) c -> p pl r c", r=4)
        v = v[:, 4 * grp:4 * grp + 4]
        d4 = dst[:, :].rearrange("p (pl r c) -> p pl r c", pl=4, r=4)
        self.nc.sync.dma_start(out=d4, in_=v)

    def mm_rowpair2(self, PS, W4, XT, pl, r4):
        x3 = XT[:, :].rearrange("p (pl r c) -> p pl r c", pl=4, r=4)
        out = PS[:, :].rearrange("p (pl c) -> p pl c", pl=4)
        self.nc.tensor.matmul(out[:, pl], W4[:, :], x3[:, pl, r4],
                              start=(r4 == 0), stop=(r4 == 3))

    def store_ll2_group(self, grp, C2):
        dst = self.ll2d[4 * grp:4 * grp + 4].rearrange("pl r c -> r pl c")
        src = C2[:, :].rearrange("p (pl c) -> p pl c", pl=4)
        self.nc.sync.dma_start(out=dst, in_=src)

    def load_ll2_batch(self, t, dst):
        src = self.ll2d[BPL * t:BPL * (t + 1)].rearrange(
            "pl (blk j) c -> (pl blk) (j c)", blk=NBLK)
        self.nc.sync.dma_start(out=dst[:, :], in_=src)

    def repack_tail(self, t, ll, tail):
        # [128, 32] sbuf -> dram bounce -> tail[16t:16t+16, :] ([16, 256])
        self.nc.sync.dma_start(out=self.bounce[:, :], in_=ll[:, :])
        src = self.bounce.rearrange("(pl b) j -> pl (b j)", b=NBLK)
        self.nc.sync.dma_start(out=tail[t * BPL:(t + 1) * BPL, :], in_=src)

    def store_outb(self, t, outb):
        self.nc.sync.dma_start(
            out=self.outblk[t * 128:(t + 1) * 128, :], in_=outb[:, :])

    def store_outt(self, outt):
        self.nc.sync.dma_start(out=self.outtail[:, :], in_=outt[:, :])

    def load_tmap(self, dst):
        self.nc.sync.dma_start(out=dst[:, :], in_=self.tmap_dram[:, :])


def build_program(h, g, opts=None):
    """Builds the single-core SPMD Tile program. Returns compiled nc."""
    from contextlib import ExitStack
    import concourse.bacc as bacc
    import concourse.tile as tile
    from concourse import mybir

    opts = opts or {}
    nc = bacc.Bacc("TRN2", target_bir_lowering=False, debug=False,
                   num_devices=NCORES)
    xs = nc.dram_tensor("xs", [PPC, 512, 512], mybir.dt.bfloat16,
                        kind="ExternalInput").ap()
    tmap = nc.dram_tensor("tmap", [PPC, 126], mybir.dt.bfloat16,
                          kind="ExternalInput").ap()
    outblk = nc.dram_tensor("out_blk", [NBATCH * 128, BLK_FLOATS],
                            mybir.dt.bfloat16, kind="ExternalOutput").ap()
    outtail = nc.dram_tensor("out_tail", [PPC, TAIL_FLOATS],
                             mybir.dt.float32, kind="ExternalOutput").ap()
    bounce = nc.dram_tensor("bounce", [128, 32], mybir.dt.bfloat16).ap()
    w4 = nc.dram_tensor("w4", [128, 128], mybir.dt.bfloat16,
                        kind="ExternalInput").ap()
    ll2d = nc.dram_tensor("ll2d", [PPC, 128, 128], mybir.dt.bfloat16).ap()

    with ExitStack() as ctx:
        tc = ctx.enter_context(tile.TileContext(nc, trace_sim=False))
        pools = {
            'work': ctx.enter_context(
                tc.tile_pool(name="work", bufs=opts.get('work_bufs', 2))),
            'xt': ctx.enter_context(
                tc.tile_pool(name="xt", bufs=opts.get('xt_bufs', 2))),
            'big': ctx.enter_context(
                tc.tile_pool(name="big", bufs=opts.get('big_bufs', 4))),
            'chain': ctx.enter_context(
                tc.tile_pool(name="chain", bufs=opts.get('chain_bufs', 8))),
            'persist': ctx.enter_context(tc.tile_pool(name="persist", bufs=1)),
            'psum': ctx.enter_context(
                tc.tile_pool(name="psum", bufs=2, space="PSUM")),
        }
        be = BassBE(tc, pools, xs, tmap, outblk, outtail, bounce,
                    w4_ap=w4, ll2d_ap=ll2d, opts=opts)
        for _ in range(opts.get('repeat', 1)):
            emit_core(be, h, g)
    nc.compile()
    return nc


# ---- public entry ---------------------------------------------------------
_CACHE = {}


def kernel(x, h, g):
    import ml_dtypes
    x = np.asarray(x)
    h = np.asarray(h, np.float32)
    g = np.asarray(g, np.float32)
    B, C = x.shape[0], x.shape[1]

    from concourse.bass_utils import run_bass_kernel_spmd

    key = (h.tobytes(), g.tobytes())
    if key not in _CACHE:
        nc = build_program(h, g, {'xt_bufs': 4, 'work_bufs': 3, 'big_bufs': 4, 'c_batches': ()})
        tmap_row = build_tail_consts(h, g, tail_scale(h, g))
        tmap = np.ascontiguousarray(
            np.broadcast_to(tmap_row, (PPC, 126))).astype(ml_dtypes.bfloat16)
        perm = build_perm()
        _CACHE[key] = (nc, tmap, perm)
    nc, tmap, perm = _CACHE[key]

    planes = np.ascontiguousarray(x.astype(ml_dtypes.bfloat16)
                                  ).reshape(NPLANES, 512, 512)
    w4 = np.ascontiguousarray(w4_matrix().astype(ml_dtypes.bfloat16))
    in_maps = [{"xs": planes[k * PPC:(k + 1) * PPC], "tmap": tmap, "w4": w4}
               for k in range(NCORES)]
    res = run_bass_kernel_spmd(nc, in_maps, list(range(NCORES)))
    global LAST_EXEC_NS
    LAST_EXEC_NS = getattr(res, 'exec_time_ns', None)
    out = np.empty((NPLANES, 9681), np.float32)
    for k in range(NCORES):
        out[k * PPC:(k + 1) * PPC] = gather_host(
            res.results[k]["out_blk"], res.results[k]["out_tail"], perm)
    return out.reshape(B, C, 9681)



# revision 48
# speedup vs baseline: 1.2018x; 1.0790x over previous
"""Contourlet transform kernel for 8 Trainium2 NeuronCores.

Input x: [16, 32, 512, 512] f32 -> output [16, 32, 9681] f32.

Strategy: 512 independent (b,c) planes, 64 per core, 4 batches of 16.
Each plane is split into 8 row-blocks of 64 rows; SBUF partition =
(plane_in_batch, block), plane data lives in the free dimension, so both
row and column 2-tap DWT passes are strided free-dim scalar_tensor_tensor
ops on the vector engine (no transposes anywhere).

Every 2-tap pass computes (a * (f1/f0) + b), i.e. the true output divided
by f0.  The dropped factors accumulate multiplicatively down the cascade;
kept subbands are fixed up by a single scaled-copy on the scalar engine
into the output staging tile.  Once the LL chain reaches 16x16 the block
layout runs out of rows, so planes are repacked to one-plane-per-partition
([64, 256]) and the remaining levels run there; everything below 2x2 is a
rank-1 linear map of the 1x1 LL value, applied as one tensor_scalar op
with 126 host-precomputed constants.

The device writes a packed layout (OUT_BLK [512,1176] + OUT_TAIL [64,273]
per core); the host gather applies a fixed permutation per plane.
"""

import numpy as np

INV_SQRT2 = 0.7071067811865476

# ---- fixed geometry -------------------------------------------------------
NPLANES = 512          # 16*32
NCORES = 8
PPC = 64               # planes per core
NBATCH = 4             # batches per core
BPL = 16               # planes per batch
NBLK = 8               # row-blocks per plane
ROWS_PER_BLK = 64      # 512 / NBLK
NSC = 8                # L1 sub-chunks per batch
SC_ROWS = 8            # rows per sub-chunk per block

# per-partition offsets of the scale0 l=0,1,2 subband regions in OUT_BLK
LOFF = [0, 896, 1120]          # 7*128, 7*32, 7*8
BLK_FLOATS = 1176              # per-partition OUT_BLK floats
# OUT_TAIL per-plane offsets
TOFF = {3: 0, 4: 112, 5: 140}  # 7*16, 7*4, 7*1
TCONST = 147                   # 126 map outputs
TAIL_FLOATS = 273


# ---- backends -------------------------------------------------------------
class NpTile:
    """numpy [P, F] tile with bass-AP-like 3-d reshaping."""

    def __init__(self, arr):
        self.arr = arr

    def __getitem__(self, key):
        return self.arr[key]

    def __setitem__(self, key, val):
        self.arr[key] = val


def w4_matrix():
    """[128, 128] identity: partition p holds rows 4p..4p+3 of a plane; 4
    accumulating identity matmuls (one per row slot) sum them into PSUM
    row p — rowpair^2 on the tensor engine with 4KB input-DMA runs."""
    return np.eye(128, dtype=np.float32)


class NumpyBE:
    """Numpy mirror of the device op plan (1 core)."""

    def __init__(self, xs, h, g, tmap):
        # xs: [64, 512, 512] planes for this core
        self.xs, self.h, self.g = xs, h, g
        self.tmap = tmap  # [126]
        self.w4 = w4_matrix()
        self.ll2d = np.zeros((PPC, 128, 128), np.float32)
        self.out_blk = np.zeros((NBATCH * 128, BLK_FLOATS), np.float32)
        self.out_tail = np.zeros((PPC, TAIL_FLOATS), np.float32)

    def alloc(self, name, shape):
        return NpTile(np.zeros(shape, np.float32))

    @staticmethod
    def r3(tile, cols, sub=None):
        """view tile (or its free-slice sub=(start,len)) as [P, rows, cols]"""
        arr = tile.arr if isinstance(tile, NpTile) else tile
        if sub is not None:
            arr = arr[:, sub[0]:sub[0] + sub[1]]
        P, F = arr.shape
        return arr.reshape(P, F // cols, cols)

    def stt(self, out, a, s, b):
        out[...] = a * np.float32(s) + b

    def tt(self, out, a, b):
        out[...] = a + b

    def copy(self, out, inp):
        out[...] = inp

    def scale_copy(self, out, inp, s):
        out[...] = inp * np.float32(s)

    def ts_mul(self, out, a, col):
        out[...] = a * col  # col: [P,1]

    def load_x_chunk(self, t, sc, dst, nsc=NSC):
        # dst [128, sc_rows*512]: partition (pl, blk) <- plane 16t+pl,
        # rows blk*64 + sc*sc_rows .. +sc_rows, all 512 cols
        sc_rows = ROWS_PER_BLK // nsc
        x = self.xs[t * BPL:(t + 1) * BPL]  # [16, 512, 512]
        v = x.reshape(BPL, NBLK, nsc, sc_rows, 512)[:, :, sc]
        dst.arr[...] = v.reshape(128, sc_rows * 512)

    # ---- plan C: TensorE phase A -----------------------------------------
    def alloc_psum(self, name, shape):
        return self.alloc(name, shape)

    def load_w4(self, dst):
        dst.arr[...] = self.w4.reshape(128, -1)

    def load_x_group(self, grp, dst):
        # dst [128, 4*4*512]: partition p = 4-row group; free (pl, r4, col)
        x = self.xs[4 * grp:4 * grp + 4]           # [4, 512, 512]
        v = x.reshape(4, 128, 4, 512).transpose(1, 0, 2, 3)
        dst.arr[...] = v.reshape(128, -1)

    def mm_rowpair2(self, PS, W4, XT, pl, r4):
        x3 = XT.arr.reshape(128, 4, 4, 512)
        acc = W4.arr.T @ x3[:, pl, r4]               # [128, 512]
        ps = PS.arr.reshape(128, 4, 512)
        if r4 == 0:
            ps[:, pl] = acc
        else:
            ps[:, pl] += acc

    def store_ll2_group(self, grp, C2):
        self.ll2d[4 * grp:4 * grp + 4] = \
            C2.arr.reshape(128, 4, 128).transpose(1, 0, 2)

    def load_ll2_batch(self, t, dst):
        src = self.ll2d[BPL * t:BPL * (t + 1)]     # [16, 128, 128]
        dst.arr[...] = src.reshape(BPL, NBLK, 16 * 128).reshape(128, 16 * 128)

    def repack_tail(self, t, ll, tail):
        # ll [128, 32] -> tail[16t:16t+16, :]: plane-major 16x16
        tail.arr[t * BPL:(t + 1) * BPL] = ll.arr.reshape(BPL, NBLK * 32)

    def store_outb(self, t, outb):
        self.out_blk[t * 128:(t + 1) * 128] = outb.arr

    def store_outt(self, outt):
        self.out_tail[...] = outt.arr

    def load_tmap(self, dst):
        dst.arr[...] = np.broadcast_to(self.tmap, (PPC, 126))


# ---- shared op plan -------------------------------------------------------
def emit_direction(be, LL, R, S, l, s, dst_tile, dst_off, P, h, g):
    """One directional decomposition: dwt2(LL, h[l]) -> LL,LH,HL,HH then
    g-decompositions keeping (A1,H1,V1,A2,H2,H3,D3) into dst at dst_off.
    LL: [P, R*S] tile viewed as (R rows, S cols) per partition.
    Returns (LL_next tile [P, (R/2)*(S/2)], new scale)."""
    f0, f1 = float(h[l, 0]), float(h[l, 1])
    g0, g1 = float(g[l, 0]), float(g[l, 1])
    rh, rg = f1 / f0, g1 / g0
    S2, R2 = S // 2, R // 2
    m = S // 4
    R4 = R // 4 if R >= 4 else 1   # rows/partition of kept bands
    L3 = be.r3(LL, S)

    CL = be.alloc("cl", [P, R * S2])
    CH = be.alloc("ch", [P, R * S2])
    be.stt(be.r3(CL, S2), L3[:, :, 0::2], rh, L3[:, :, 1::2])
    be.stt(be.r3(CH, S2), L3[:, :, 1::2], -rh, L3[:, :, 0::2])

    C3L, C3H = be.r3(CL, S2), be.r3(CH, S2)
    LLn = be.alloc("lln", [P, R2 * S2])
    LH = be.alloc("lh", [P, R2 * S2])
    HL = be.alloc("hl", [P, R2 * S2])
    HH = be.alloc("hh", [P, R2 * S2])
    be.stt(be.r3(LLn, S2), C3L[:, 0::2, :], rh, C3L[:, 1::2, :])
    be.stt(be.r3(LH, S2), C3L[:, 1::2, :], -rh, C3L[:, 0::2, :])
    be.stt(be.r3(HL, S2), C3H[:, 0::2, :], rh, C3H[:, 1::2, :])
    be.stt(be.r3(HH, S2), C3H[:, 1::2, :], -rh, C3H[:, 0::2, :])

    # g-stage on bands of size S2 (R2 rows/partition); kept bands mxm, R4 rows
    q = R4 * m                       # floats per kept band per partition
    SCR = be.alloc("scr", [P, 7 * q])

    def scr(i):
        return be.r3(SCR, m, sub=(i * q, q))

    GL = be.alloc("gl", [P, R2 * m])
    GH = be.alloc("gh", [P, R2 * m])

    # LH -> A1 (row-lo col-lo), H1 (row-hi col-lo), V1 (row-lo col-hi)
    B3 = be.r3(LH, S2)
    be.stt(be.r3(GL, m), B3[:, :, 0::2], rg, B3[:, :, 1::2])
    be.stt(be.r3(GH, m), B3[:, :, 1::2], -rg, B3[:, :, 0::2])
    G3L, G3H = be.r3(GL, m), be.r3(GH, m)
    be.stt(scr(0), G3L[:, 0::2, :], rg, G3L[:, 1::2, :])
    be.stt(scr(1), G3L[:, 1::2, :], -rg, G3L[:, 0::2, :])
    be.stt(scr(2), G3H[:, 0::2, :], rg, G3H[:, 1::2, :])

    # HL -> A2 (row-lo col-lo), H2 (row-hi col-lo): col-lo branch only
    GL2 = be.alloc("gl2", [P, R2 * m])
    B3 = be.r3(HL, S2)
    be.stt(be.r3(GL2, m), B3[:, :, 0::2], rg, B3[:, :, 1::2])
    G3L = be.r3(GL2, m)
    be.stt(scr(3), G3L[:, 0::2, :], rg, G3L[:, 1::2, :])
    be.stt(scr(4), G3L[:, 1::2, :], -rg, G3L[:, 0::2, :])

    # HH -> H3 (row-hi col-lo), D3 (row-hi col-hi)
    GL3 = be.alloc("gl3", [P, R2 * m])
    GH3 = be.alloc("gh3", [P, R2 * m])
    B3 = be.r3(HH, S2)
    be.stt(be.r3(GL3, m), B3[:, :, 0::2], rg, B3[:, :, 1::2])
    be.stt(be.r3(GH3, m), B3[:, :, 1::2], -rg, B3[:, :, 0::2])
    G3L, G3H = be.r3(GL3, m), be.r3(GH3, m)
    be.stt(scr(5), G3L[:, 1::2, :], -rg, G3L[:, 0::2, :])
    be.stt(scr(6), G3H[:, 1::2, :], -rg, G3H[:, 0::2, :])

    s_band = s * (f0 * f0) * (g0 * g0)
    be.scale_copy(dst_tile[:, dst_off:dst_off + 7 * q], SCR[:, :], s_band)
    return LLn, s * f0 * f0


def emit_core(be, h, g):
    """Full per-core program."""
    c = INV_SQRT2
    TAILLL = be.alloc("tailll", [PPC, 256])
    TMAP = be.alloc("tmap", [PPC, 126])
    be.load_tmap(TMAP)
    W4 = be.alloc("w4t", [128, 128])
    be.load_w4(W4)

    # ---- phase A: fused L1+L2 Haar, LL2 = colpair^2(rowpair^2(X)) --------
    # Groups of 4 planes; rowpair^2 on TensorE (4 accumulating identity
    # matmuls, partition = 4-row group -> 4KB DMA runs), colpair^2 on DVE
    # via an SBUF bounce on the scalar engine; LL2 bounced to DRAM
    # plane-major so the blocked reload gets 4KB descriptors.
    def phase_a_group(grp):
        XT = be.alloc("xg", [128, 4 * 4 * 512])
        be.load_x_group(grp, XT)
        PS = be.alloc_psum("ps", [128, 4 * 512])
        for pl in range(4):
            for r4 in range(4):
                be.mm_rowpair2(PS, W4, XT, pl, r4)
        # DVE cannot read both TT operands from PSUM; bounce through
        # SBUF on the (otherwise idle) scalar engine, f32 -> bf16.
        PG = be.alloc("pg", [128, 4 * 512])
        be.copy(PG[:, :], PS[:, :])
        P3 = be.r3(PG, 512)
        C1 = be.alloc("c1g", [128, 4 * 256])
        be.tt(be.r3(C1, 256), P3[:, :, 0::2], P3[:, :, 1::2])
        C13 = be.r3(C1, 256)
        C2 = be.alloc("c2g", [128, 4 * 128])
        be.tt(be.r3(C2, 128), C13[:, :, 0::2], C13[:, :, 1::2])
        be.store_ll2_group(grp, C2)

    # Hybrid phase A: batches in C_BATCHES go through the TensorE path
    # (row-major groups -> matmul rowpair^2 -> ll2d bounce); the rest use
    # the blocked all-DVE path (8KB-descriptor DMA, bf16 2x row passes)
    # whose big independent ops also keep the vector engine's issue gaps
    # filled under the direction chains.
    c_batches = tuple(be.opts.get('c_batches', (0, 1))) if hasattr(be, 'opts') \
        else (0, 1)
    # Batch-major main loop (the empirically best-scheduling structure):
    # per batch, the 8 big fused-Haar chunk chains provide abundant
    # independent DVE work that fills the direction chains' issue gaps.
    s_tail = None
    for t in range(NBATCH):
        OUTB = be.alloc("outb", [128, BLK_FLOATS])
        LL2 = be.alloc("ll2", [128, 16 * 128])
        if t in c_batches:
            for grp in range(4 * t, 4 * t + 4):
                phase_a_group(grp)
            be.load_ll2_batch(t, LL2)
        else:
            for sc in range(NSC):
                XT = be.alloc("xt", [128, SC_ROWS * 512])
                be.load_x_chunk(t, sc, XT, NSC)
                X3 = be.r3(XT, 512)
                R1 = be.alloc("r1", [128, 4 * 512])
                be.tt(be.r3(R1, 512), X3[:, 0::2, :], X3[:, 1::2, :])
                R13 = be.r3(R1, 512)
                R2 = be.alloc("r2", [128, 2 * 512])
                be.tt(be.r3(R2, 512), R13[:, 0::2, :], R13[:, 1::2, :])
                R23 = be.r3(R2, 512)
                C1 = be.alloc("c1", [128, 2 * 256])
                be.tt(be.r3(C1, 256), R23[:, :, 0::2], R23[:, :, 1::2])
                C13 = be.r3(C1, 256)
                ll2_slice = be.r3(LL2, 128)[:, 2 * sc:2 * sc + 2, :]
                be.tt(ll2_slice, C13[:, :, 0::2], C13[:, :, 1::2])

        LL, s, R, S = LL2, c ** 4, 16, 128
        for l in range(3):
            LL, s = emit_direction(be, LL, R, S, l, s, OUTB, LOFF[l], 128, h, g)
            R, S = R // 2, S // 2
        # LL now [128, 2*16] = 16x16 plane spread over 8 blocks
        be.repack_tail(t, LL, TAILLL)
        be.store_outb(t, OUTB)
        s_tail = s

    # ---- tail: plane-major [64 planes, ...] --------------------------------
    OUTT = be.alloc("outt", [PPC, TAIL_FLOATS])
    LL, s, R, S = TAILLL, s_tail, 16, 16
    for l in (3, 4, 5):
        LL, s = emit_direction(be, LL, R, S, l, s, OUTT, TOFF[l], PPC, h, g)
        R, S = R // 2, S // 2
    # LL: [64, 4] = 2x2.  scale1 Haar -> 1x1
    CT = be.alloc("ct", [PPC, 2])
    L3 = be.r3(LL, 2)
    be.stt(be.r3(CT, 1), L3[:, :, 0:1], 1.0, L3[:, :, 1:2])
    LL11 = be.alloc("ll11", [PPC, 1])
    be.stt(LL11[:, 0:1], CT[:, 0:1], 1.0, CT[:, 1:2])
    # scales 1-3 for all 6 directions: rank-1 map of LL11 (consts incl. s)
    be.ts_mul(OUTT[:, TCONST:TCONST + 126], TMAP[:, :], LL11[:, 0:1])
    be.store_outt(OUTT)
    return s * c * c  # scale of LL11 (true = s11 * raw); informational


# ---- host-side constants --------------------------------------------------
def _dwt2_np(x, f0, f1):
    def dwt_last(x):
        n = x.shape[-1]
        m = (n + 1) // 2
        xe = np.pad(x, [(0, 0)] * (x.ndim - 1) + [(1, 1)], mode='edge')
        a = xe[..., 1:2 * m + 1:2]
        b = xe[..., 2:2 * m + 2:2]
        return f1 * a + f0 * b, f0 * a - f1 * b

    lo, hi = dwt_last(x)
    lo, hi = np.swapaxes(lo, -1, -2), np.swapaxes(hi, -1, -2)
    ll, lh = dwt_last(lo)
    hl, hh = dwt_last(hi)
    sw = lambda t: np.swapaxes(t, -1, -2)
    return sw(ll), sw(lh), sw(hl), sw(hh)


def build_tail_consts(h, g, s11):
    """126 constants: scales 1-3 outputs as multiples of the raw 1x1 LL."""
    c = INV_SQRT2
    h = np.asarray(h, np.float64)
    g = np.asarray(g, np.float64)
    LL = np.ones((1, 1))
    vals = []
    for k in range(1, 4):
        if k > 1:
            LL, _, _, _ = _dwt2_np(LL, c, c)
        for l in range(6):
            LL, LH, HL, HH = _dwt2_np(LL, h[l, 0], h[l, 1])
            A1, H1, V1, _ = _dwt2_np(LH, g[l, 0], g[l, 1])
            A2, H2, _, _ = _dwt2_np(HL, g[l, 0], g[l, 1])
            _, H3, _, D3 = _dwt2_np(HH, g[l, 0], g[l, 1])
            for sb in (A1, H1, V1, A2, H2, H3, D3):
                vals.append(float(sb[0, 0]))
    return (np.asarray(vals, np.float64) * s11).astype(np.float32)


def tail_scale(h, g):
    """scale s11 of the raw 1x1 LL value (true = s11 * raw)."""
    c = INV_SQRT2
    s = c ** 4  # L1 + L2 Haar drops
    for l in range(6):
        s *= float(h[l, 0]) ** 2
    return s * c * c  # scale1 Haar drops


def build_perm():
    """perm[ref_pos] = index into per-plane packed vector
    v = concat(OUT_BLK rows for blocks 0..7 (8*1176), OUT_TAIL row (273))."""
    perm = np.empty(9681, np.int64)
    off = 0
    for l, m in enumerate((32, 16, 8)):
        rpb = m // NBLK
        loff = LOFF[l]
        for sb in range(7):
            for row in range(m):
                blk, rl = divmod(row, rpb)
                base = blk * BLK_FLOATS + loff + sb * rpb * m + rl * m
                perm[off + sb * m * m + row * m:off + sb * m * m + (row + 1) * m] = \
                    np.arange(base, base + m)
        off += 7 * m * m
    tail_base = NBLK * BLK_FLOATS
    for l, m in ((3, 4), (4, 2), (5, 1)):
        n = 7 * m * m
        perm[off:off + n] = tail_base + TOFF[l] + np.arange(n)
        off += n
    perm[off:off + 126] = tail_base + TCONST + np.arange(126)
    assert off + 126 == 9681
    return perm


def gather_host(out_blk, out_tail, perm):
    """[512,1176],[64,273] per core -> [64, 9681] in reference order."""
    v = np.concatenate(
        [out_blk.astype(np.float32).reshape(
            NBATCH, BPL, NBLK * BLK_FLOATS).reshape(PPC, -1),
         np.asarray(out_tail, np.float32)], axis=1)
    return v[:, perm]


# ---- device backend -------------------------------------------------------
class BassBE:
    """Emits the op plan as a Tile program."""

    def __init__(self, tc, pools, xs_ap, tmap_ap, outblk_ap, outtail_ap,
                 dram_bounce, w4_ap=None, ll2d_ap=None, opts=None):
        self.opts = opts or {}
        self.tc = tc
        self.nc = tc.nc
        self.pools = pools
        self.xs = xs_ap          # [64, 512, 512] dram
        self.tmap_dram = tmap_ap  # [64, 126] dram
        self.outblk = outblk_ap  # [512, 1176] dram
        self.outtail = outtail_ap  # [64, 273] dram
        self.bounce = dram_bounce  # [128, 32] dram scratch
        self.w4_dram = w4_ap     # [128, 32] dram
        self.ll2d = ll2d_ap      # [64, 128, 128] dram scratch

    def alloc(self, name, shape):
        from concourse import mybir
        if name in ('tailll', 'tmap', 'w4t'):
            pool = self.pools['persist']
        elif name == 'll2':
            pool = self.pools['big']
        elif name in ('xt', 'xg'):
            pool = self.pools['xt']
        else:
            pool = self.pools['work']
        dt = (mybir.dt.float32 if name in ('outt', 'll11')
              else mybir.dt.bfloat16)
        return pool.tile(list(shape), dt, tag=name, name=name)

    @staticmethod
    def r3(tile, cols, sub=None):
        ap = tile[:, :] if not hasattr(tile, 'ap') else tile[:, :]
        if sub is not None:
            ap = ap[:, sub[0]:sub[0] + sub[1]]
        P, F = ap.shape
        return ap.rearrange("p (r c) -> p r c", c=cols)

    def stt(self, out, a, s, b):
        from concourse import mybir
        self.nc.vector.scalar_tensor_tensor(
            out=out, in0=a, scalar=float(s), in1=b,
            op0=mybir.AluOpType.mult, op1=mybir.AluOpType.add)

    def tt(self, out, a, b):
        self.nc.vector.tensor_add(out, a, b)

    def copy(self, out, inp):
        self.nc.scalar.copy(out, inp)

    def scale_copy(self, out, inp, s):
        if self.opts.get('comp_engine', 'scalar') == 'vector':
            from concourse import mybir
            self.nc.vector.tensor_scalar(
                out=out, in0=inp, scalar1=float(s), scalar2=None,
                op0=mybir.AluOpType.mult)
        else:
            self.nc.scalar.mul(out, inp, float(s))

    def ts_mul(self, out, a, col):
        from concourse import mybir
        self.nc.vector.tensor_scalar(
            out=out, in0=a, scalar1=col, scalar2=None,
            op0=mybir.AluOpType.mult)

    def load_x_chunk(self, t, sc, dst, nsc=NSC):
        v = self.xs.rearrange("pl (blk s r) c -> pl blk s r c", blk=NBLK, s=nsc)
        v = v[t * BPL:(t + 1) * BPL, :, sc]
        v = v.rearrange("pl blk r c -> (pl blk) (r c)")
        self.nc.sync.dma_start(out=dst[:, :], in_=v)

    # ---- plan C: TensorE phase A -----------------------------------------
    def alloc_psum(self, name, shape):
        from concourse import mybir
        return self.pools['psum'].tile(list(shape), mybir.dt.float32,
                                       tag=name, name=name)

    def load_w4(self, dst):
        self.nc.sync.dma_start(out=dst[:, :], in_=self.w4_dram)

    def load_x_group(self, grp, dst):
        v = self.xs.rearrange("pl (p r) c -> p pl r c", r=4)
        v = v[:, 4 * grp:4 * grp + 4]
        d4 = dst[:, :].rearrange("p (pl r c) -> p pl r c", pl=4, r=4)
        self.nc.sync.dma_start(out=d4, in_=v)

    def mm_rowpair2(self, PS, W4, XT, pl, r4):
        x3 = XT[:, :].rearrange("p (pl r c) -> p pl r c", pl=4, r=4)
        out = PS[:, :].rearrange("p (pl c) -> p pl c", pl=4)
        self.nc.tensor.matmul(out[:, pl], W4[:, :], x3[:, pl, r4],
                              start=(r4 == 0), stop=(r4 == 3))

    def store_ll2_group(self, grp, C2):
        dst = self.ll2d[4 * grp:4 * grp + 4].rearrange("pl r c -> r pl c")
        src = C2[:, :].rearrange("p (pl c) -> p pl c", pl=4)
        self.nc.sync.dma_start(out=dst, in_=src)

    def load_ll2_batch(self, t, dst):
        src = self.ll2d[BPL * t:BPL * (t + 1)].rearrange(
            "pl (blk j) c -> (pl blk) (j c)", blk=NBLK)
        self.nc.sync.dma_start(out=dst[:, :], in_=src)

    def repack_tail(self, t, ll, tail):
        # [128, 32] sbuf -> dram bounce -> tail[16t:16t+16, :] ([16, 256])
        self.nc.sync.dma_start(out=self.bounce[:, :], in_=ll[:, :])
        src = self.bounce.rearrange("(pl b) j -> pl (b j)", b=NBLK)
        self.nc.sync.dma_start(out=tail[t * BPL:(t + 1) * BPL, :], in_=src)

    def store_outb(self, t, outb):
        self.nc.sync.dma_start(
            out=self.outblk[t * 128:(t + 1) * 128, :], in_=outb[:, :])

    def store_outt(self, outt):
        self.nc.sync.dma_start(out=self.outtail[:, :], in_=outt[:, :])

    def load_tmap(self, dst):
        self.nc.sync.dma_start(out=dst[:, :], in_=self.tmap_dram[:, :])


def build_program(h, g, opts=None):
    """Builds the single-core SPMD Tile program. Returns compiled nc."""
    from contextlib import ExitStack
    import concourse.bacc as bacc
    import concourse.tile as tile
    from concourse import mybir

    opts = opts or {}
    nc = bacc.Bacc("TRN2", target_bir_lowering=False, debug=False,
                   num_devices=NCORES)
    xs = nc.dram_tensor("xs", [PPC, 512, 512], mybir.dt.bfloat16,
                        kind="ExternalInput").ap()
    tmap = nc.dram_tensor("tmap", [PPC, 126], mybir.dt.bfloat16,
                          kind="ExternalInput").ap()
    outblk = nc.dram_tensor("out_blk", [NBATCH * 128, BLK_FLOATS],
                            mybir.dt.bfloat16, kind="ExternalOutput").ap()
    outtail = nc.dram_tensor("out_tail", [PPC, TAIL_FLOATS],
                             mybir.dt.float32, kind="ExternalOutput").ap()
    bounce = nc.dram_tensor("bounce", [128, 32], mybir.dt.bfloat16).ap()
    w4 = nc.dram_tensor("w4", [128, 128], mybir.dt.bfloat16,
                        kind="ExternalInput").ap()
    ll2d = nc.dram_tensor("ll2d", [PPC, 128, 128], mybir.dt.bfloat16).ap()

    with ExitStack() as ctx:
        tc = ctx.enter_context(tile.TileContext(nc, trace_sim=False))
        pools = {
            'work': ctx.enter_context(
                tc.tile_pool(name="work", bufs=opts.get('work_bufs', 2))),
            'xt': ctx.enter_context(
                tc.tile_pool(name="xt", bufs=opts.get('xt_bufs', 2))),
            'big': ctx.enter_context(
                tc.tile_pool(name="big", bufs=opts.get('big_bufs', 4))),
            'chain': ctx.enter_context(
                tc.tile_pool(name="chain", bufs=opts.get('chain_bufs', 8))),
            'persist': ctx.enter_context(tc.tile_pool(name="persist", bufs=1)),
            'psum': ctx.enter_context(
                tc.tile_pool(name="psum", bufs=2, space="PSUM")),
        }
        be = BassBE(tc, pools, xs, tmap, outblk, outtail, bounce,
                    w4_ap=w4, ll2d_ap=ll2d, opts=opts)
        for _ in range(opts.get('repeat', 1)):
            emit_core(be, h, g)
    nc.compile()
    return nc


# ---- public entry ---------------------------------------------------------
_CACHE = {}


def kernel(x, h, g):
    import ml_dtypes
    x = np.asarray(x)
    h = np.asarray(h, np.float32)
    g = np.asarray(g, np.float32)
    B, C = x.shape[0], x.shape[1]

    from concourse.bass_utils import run_bass_kernel_spmd

    key = (h.tobytes(), g.tobytes())
    if key not in _CACHE:
        nc = build_program(h, g, {'xt_bufs': 4, 'work_bufs': 2, 'big_bufs': 1, 'c_batches': ()})
        tmap_row = build_tail_consts(h, g, tail_scale(h, g))
        tmap = np.ascontiguousarray(
            np.broadcast_to(tmap_row, (PPC, 126))).astype(ml_dtypes.bfloat16)
        perm = build_perm()
        _CACHE[key] = (nc, tmap, perm)
    nc, tmap, perm = _CACHE[key]

    planes = np.ascontiguousarray(x.astype(ml_dtypes.bfloat16)
                                  ).reshape(NPLANES, 512, 512)
    w4 = np.ascontiguousarray(w4_matrix().astype(ml_dtypes.bfloat16))
    in_maps = [{"xs": planes[k * PPC:(k + 1) * PPC], "tmap": tmap, "w4": w4}
               for k in range(NCORES)]
    res = run_bass_kernel_spmd(nc, in_maps, list(range(NCORES)))
    global LAST_EXEC_NS
    LAST_EXEC_NS = getattr(res, 'exec_time_ns', None)
    out = np.empty((NPLANES, 9681), np.float32)
    for k in range(NCORES):
        out[k * PPC:(k + 1) * PPC] = gather_host(
            res.results[k]["out_blk"], res.results[k]["out_tail"], perm)
    return out.reshape(B, C, 9681)



# revision 50
# speedup vs baseline: 1.2398x; 1.0316x over previous
"""Contourlet transform kernel for 8 Trainium2 NeuronCores.

Input x: [16, 32, 512, 512] f32 -> output [16, 32, 9681] f32.

Strategy: 512 independent (b,c) planes, 64 per core, 4 batches of 16.
Each plane is split into 8 row-blocks of 64 rows; SBUF partition =
(plane_in_batch, block), plane data lives in the free dimension, so both
row and column 2-tap DWT passes are strided free-dim scalar_tensor_tensor
ops on the vector engine (no transposes anywhere).

Every 2-tap pass computes (a * (f1/f0) + b), i.e. the true output divided
by f0.  The dropped factors accumulate multiplicatively down the cascade;
kept subbands are fixed up by a single scaled-copy on the scalar engine
into the output staging tile.  Once the LL chain reaches 16x16 the block
layout runs out of rows, so planes are repacked to one-plane-per-partition
([64, 256]) and the remaining levels run there; everything below 2x2 is a
rank-1 linear map of the 1x1 LL value, applied as one tensor_scalar op
with 126 host-precomputed constants.

The device writes a packed layout (OUT_BLK [512,1176] + OUT_TAIL [64,273]
per core); the host gather applies a fixed permutation per plane.
"""

import numpy as np

INV_SQRT2 = 0.7071067811865476

# ---- fixed geometry -------------------------------------------------------
NPLANES = 512          # 16*32
NCORES = 8
PPC = 64               # planes per core
NBATCH = 4             # batches per core
BPL = 16               # planes per batch
NBLK = 8               # row-blocks per plane
ROWS_PER_BLK = 64      # 512 / NBLK
NSC = 8                # L1 sub-chunks per batch
SC_ROWS = 8            # rows per sub-chunk per block

# per-partition offsets of the scale0 l=0,1,2 subband regions in OUT_BLK
LOFF = [0, 896, 1120]          # 7*128, 7*32, 7*8
BLK_FLOATS = 1176              # per-partition OUT_BLK floats
# OUT_TAIL per-plane offsets
TOFF = {3: 0, 4: 112, 5: 140}  # 7*16, 7*4, 7*1
TCONST = 147                   # 126 map outputs
TAIL_FLOATS = 273


# ---- backends -------------------------------------------------------------
class NpTile:
    """numpy [P, F] tile with bass-AP-like 3-d reshaping."""

    def __init__(self, arr):
        self.arr = arr

    def __getitem__(self, key):
        return self.arr[key]

    def __setitem__(self, key, val):
        self.arr[key] = val


def w4_matrix():
    """[128, 128] identity: partition p holds rows 4p..4p+3 of a plane; 4
    accumulating identity matmuls (one per row slot) sum them into PSUM
    row p — rowpair^2 on the tensor engine with 4KB input-DMA runs."""
    return np.eye(128, dtype=np.float32)


class NumpyBE:
    """Numpy mirror of the device op plan (1 core)."""

    def __init__(self, xs, h, g, tmap):
        # xs: [64, 512, 512] planes for this core
        self.xs, self.h, self.g = xs, h, g
        self.tmap = tmap  # [126]
        self.w4 = w4_matrix()
        self.ll2d = np.zeros((PPC, 128, 128), np.float32)
        self.out_blk = np.zeros((NBATCH * 128, BLK_FLOATS), np.float32)
        self.out_tail = np.zeros((PPC, TAIL_FLOATS), np.float32)

    def alloc(self, name, shape):
        return NpTile(np.zeros(shape, np.float32))

    @staticmethod
    def r3(tile, cols, sub=None):
        """view tile (or its free-slice sub=(start,len)) as [P, rows, cols]"""
        arr = tile.arr if isinstance(tile, NpTile) else tile
        if sub is not None:
            arr = arr[:, sub[0]:sub[0] + sub[1]]
        P, F = arr.shape
        return arr.reshape(P, F // cols, cols)

    def stt(self, out, a, s, b):
        out[...] = a * np.float32(s) + b

    def tt(self, out, a, b):
        out[...] = a + b

    def copy(self, out, inp):
        out[...] = inp

    def scale_copy(self, out, inp, s):
        out[...] = inp * np.float32(s)

    def ts_mul(self, out, a, col):
        out[...] = a * col  # col: [P,1]

    def load_x_chunk(self, t, sc, dst, nsc=NSC):
        # dst [128, sc_rows*512]: partition (pl, blk) <- plane 16t+pl,
        # rows blk*64 + sc*sc_rows .. +sc_rows, all 512 cols
        sc_rows = ROWS_PER_BLK // nsc
        x = self.xs[t * BPL:(t + 1) * BPL]  # [16, 512, 512]
        v = x.reshape(BPL, NBLK, nsc, sc_rows, 512)[:, :, sc]
        dst.arr[...] = v.reshape(128, sc_rows * 512)

    # ---- plan C: TensorE phase A -----------------------------------------
    def alloc_psum(self, name, shape):
        return self.alloc(name, shape)

    def load_w4(self, dst):
        dst.arr[...] = self.w4.reshape(128, -1)

    def load_x_group(self, grp, dst):
        # dst [128, 4*4*512]: partition p = 4-row group; free (pl, r4, col)
        x = self.xs[4 * grp:4 * grp + 4]           # [4, 512, 512]
        v = x.reshape(4, 128, 4, 512).transpose(1, 0, 2, 3)
        dst.arr[...] = v.reshape(128, -1)

    def mm_rowpair2(self, PS, W4, XT, pl, r4):
        x3 = XT.arr.reshape(128, 4, 4, 512)
        acc = W4.arr.T @ x3[:, pl, r4]               # [128, 512]
        ps = PS.arr.reshape(128, 4, 512)
        if r4 == 0:
            ps[:, pl] = acc
        else:
            ps[:, pl] += acc

    def store_ll2_group(self, grp, C2):
        self.ll2d[4 * grp:4 * grp + 4] = \
            C2.arr.reshape(128, 4, 128).transpose(1, 0, 2)

    def load_ll2_batch(self, t, dst):
        src = self.ll2d[BPL * t:BPL * (t + 1)]     # [16, 128, 128]
        dst.arr[...] = src.reshape(BPL, NBLK, 16 * 128).reshape(128, 16 * 128)

    def repack_tail(self, t, ll, tail):
        # ll [128, 32] -> tail[16t:16t+16, :]: plane-major 16x16
        tail.arr[t * BPL:(t + 1) * BPL] = ll.arr.reshape(BPL, NBLK * 32)

    def store_outb(self, t, outb):
        self.out_blk[t * 128:(t + 1) * 128] = outb.arr

    def store_outt(self, outt):
        self.out_tail[...] = outt.arr

    def load_tmap(self, dst):
        dst.arr[...] = np.broadcast_to(self.tmap, (PPC, 126))


# ---- shared op plan -------------------------------------------------------
def emit_direction(be, LL, R, S, l, s, dst_tile, dst_off, P, h, g):
    """One directional decomposition: dwt2(LL, h[l]) -> LL,LH,HL,HH then
    g-decompositions keeping (A1,H1,V1,A2,H2,H3,D3) into dst at dst_off.
    LL: [P, R*S] tile viewed as (R rows, S cols) per partition.
    Returns (LL_next tile [P, (R/2)*(S/2)], new scale)."""
    f0, f1 = float(h[l, 0]), float(h[l, 1])
    g0, g1 = float(g[l, 0]), float(g[l, 1])
    rh, rg = f1 / f0, g1 / g0
    S2, R2 = S // 2, R // 2
    m = S // 4
    R4 = R // 4 if R >= 4 else 1   # rows/partition of kept bands
    L3 = be.r3(LL, S)

    CL = be.alloc("cl", [P, R * S2])
    CH = be.alloc("ch", [P, R * S2])
    be.stt(be.r3(CL, S2), L3[:, :, 0::2], rh, L3[:, :, 1::2])
    be.stt(be.r3(CH, S2), L3[:, :, 1::2], -rh, L3[:, :, 0::2])

    C3L, C3H = be.r3(CL, S2), be.r3(CH, S2)
    LLn = be.alloc("lln", [P, R2 * S2])
    LH = be.alloc("lh", [P, R2 * S2])
    HL = be.alloc("hl", [P, R2 * S2])
    HH = be.alloc("hh", [P, R2 * S2])
    be.stt(be.r3(LLn, S2), C3L[:, 0::2, :], rh, C3L[:, 1::2, :])
    be.stt(be.r3(LH, S2), C3L[:, 1::2, :], -rh, C3L[:, 0::2, :])
    be.stt(be.r3(HL, S2), C3H[:, 0::2, :], rh, C3H[:, 1::2, :])
    be.stt(be.r3(HH, S2), C3H[:, 1::2, :], -rh, C3H[:, 0::2, :])

    # g-stage on bands of size S2 (R2 rows/partition); kept bands mxm, R4 rows
    q = R4 * m                       # floats per kept band per partition
    SCR = be.alloc("scr", [P, 7 * q])

    def scr(i):
        return be.r3(SCR, m, sub=(i * q, q))

    GL = be.alloc("gl", [P, R2 * m])
    GH = be.alloc("gh", [P, R2 * m])

    # LH -> A1 (row-lo col-lo), H1 (row-hi col-lo), V1 (row-lo col-hi)
    B3 = be.r3(LH, S2)
    be.stt(be.r3(GL, m), B3[:, :, 0::2], rg, B3[:, :, 1::2])
    be.stt(be.r3(GH, m), B3[:, :, 1::2], -rg, B3[:, :, 0::2])
    G3L, G3H = be.r3(GL, m), be.r3(GH, m)
    be.stt(scr(0), G3L[:, 0::2, :], rg, G3L[:, 1::2, :])
    be.stt(scr(1), G3L[:, 1::2, :], -rg, G3L[:, 0::2, :])
    be.stt(scr(2), G3H[:, 0::2, :], rg, G3H[:, 1::2, :])

    # HL -> A2 (row-lo col-lo), H2 (row-hi col-lo): col-lo branch only
    GL2 = be.alloc("gl2", [P, R2 * m])
    B3 = be.r3(HL, S2)
    be.stt(be.r3(GL2, m), B3[:, :, 0::2], rg, B3[:, :, 1::2])
    G3L = be.r3(GL2, m)
    be.stt(scr(3), G3L[:, 0::2, :], rg, G3L[:, 1::2, :])
    be.stt(scr(4), G3L[:, 1::2, :], -rg, G3L[:, 0::2, :])

    # HH -> H3 (row-hi col-lo), D3 (row-hi col-hi)
    GL3 = be.alloc("gl3", [P, R2 * m])
    GH3 = be.alloc("gh3", [P, R2 * m])
    B3 = be.r3(HH, S2)
    be.stt(be.r3(GL3, m), B3[:, :, 0::2], rg, B3[:, :, 1::2])
    be.stt(be.r3(GH3, m), B3[:, :, 1::2], -rg, B3[:, :, 0::2])
    G3L, G3H = be.r3(GL3, m), be.r3(GH3, m)
    be.stt(scr(5), G3L[:, 1::2, :], -rg, G3L[:, 0::2, :])
    be.stt(scr(6), G3H[:, 1::2, :], -rg, G3H[:, 0::2, :])

    s_band = s * (f0 * f0) * (g0 * g0)
    be.scale_copy(dst_tile[:, dst_off:dst_off + 7 * q], SCR[:, :], s_band)
    return LLn, s * f0 * f0


def emit_core(be, h, g):
    """Full per-core program."""
    c = INV_SQRT2
    TAILLL = be.alloc("tailll", [PPC, 256])
    TMAP = be.alloc("tmap", [PPC, 126])
    be.load_tmap(TMAP)
    W4 = be.alloc("w4t", [128, 128])
    be.load_w4(W4)

    # ---- phase A: fused L1+L2 Haar, LL2 = colpair^2(rowpair^2(X)) --------
    # Groups of 4 planes; rowpair^2 on TensorE (4 accumulating identity
    # matmuls, partition = 4-row group -> 4KB DMA runs), colpair^2 on DVE
    # via an SBUF bounce on the scalar engine; LL2 bounced to DRAM
    # plane-major so the blocked reload gets 4KB descriptors.
    def phase_a_group(grp):
        XT = be.alloc("xg", [128, 4 * 4 * 512])
        be.load_x_group(grp, XT)
        PS = be.alloc_psum("ps", [128, 4 * 512])
        for pl in range(4):
            for r4 in range(4):
                be.mm_rowpair2(PS, W4, XT, pl, r4)
        # DVE cannot read both TT operands from PSUM; bounce through
        # SBUF on the (otherwise idle) scalar engine, f32 -> bf16.
        PG = be.alloc("pg", [128, 4 * 512])
        be.copy(PG[:, :], PS[:, :])
        P3 = be.r3(PG, 512)
        C1 = be.alloc("c1g", [128, 4 * 256])
        be.tt(be.r3(C1, 256), P3[:, :, 0::2], P3[:, :, 1::2])
        C13 = be.r3(C1, 256)
        C2 = be.alloc("c2g", [128, 4 * 128])
        be.tt(be.r3(C2, 128), C13[:, :, 0::2], C13[:, :, 1::2])
        be.store_ll2_group(grp, C2)

    # Hybrid phase A: batches in C_BATCHES go through the TensorE path
    # (row-major groups -> matmul rowpair^2 -> ll2d bounce); the rest use
    # the blocked all-DVE path (8KB-descriptor DMA, bf16 2x row passes)
    # whose big independent ops also keep the vector engine's issue gaps
    # filled under the direction chains.
    c_batches = tuple(be.opts.get('c_batches', (0, 1))) if hasattr(be, 'opts') \
        else (0, 1)
    # Batch-major main loop (the empirically best-scheduling structure):
    # per batch, the 8 big fused-Haar chunk chains provide abundant
    # independent DVE work that fills the direction chains' issue gaps.
    s_tail = None
    for t in range(NBATCH):
        OUTB = be.alloc("outb", [128, BLK_FLOATS])
        LL2 = be.alloc("ll2", [128, 16 * 128])
        if t in c_batches:
            for grp in range(4 * t, 4 * t + 4):
                phase_a_group(grp)
            be.load_ll2_batch(t, LL2)
        else:
            nsc = be.opts.get('nsc', NSC) if hasattr(be, 'opts') else NSC
            sc_rows = ROWS_PER_BLK // nsc
            rr = sc_rows // 4
            for sc in range(nsc):
                XT = be.alloc("xt", [128, sc_rows * 512])
                be.load_x_chunk(t, sc, XT, nsc)
                X3 = be.r3(XT, 512)
                R1 = be.alloc("r1", [128, (sc_rows // 2) * 512])
                be.tt(be.r3(R1, 512), X3[:, 0::2, :], X3[:, 1::2, :])
                R13 = be.r3(R1, 512)
                R2 = be.alloc("r2", [128, rr * 512])
                be.tt(be.r3(R2, 512), R13[:, 0::2, :], R13[:, 1::2, :])
                R23 = be.r3(R2, 512)
                C1 = be.alloc("c1", [128, rr * 256])
                be.tt(be.r3(C1, 256), R23[:, :, 0::2], R23[:, :, 1::2])
                C13 = be.r3(C1, 256)
                ll2_slice = be.r3(LL2, 128)[:, rr * sc:rr * (sc + 1), :]
                be.tt(ll2_slice, C13[:, :, 0::2], C13[:, :, 1::2])

        LL, s, R, S = LL2, c ** 4, 16, 128
        for l in range(3):
            LL, s = emit_direction(be, LL, R, S, l, s, OUTB, LOFF[l], 128, h, g)
            R, S = R // 2, S // 2
        # LL now [128, 2*16] = 16x16 plane spread over 8 blocks
        be.repack_tail(t, LL, TAILLL)
        be.store_outb(t, OUTB)
        s_tail = s

    # ---- tail: plane-major [64 planes, ...] --------------------------------
    OUTT = be.alloc("outt", [PPC, TAIL_FLOATS])
    LL, s, R, S = TAILLL, s_tail, 16, 16
    for l in (3, 4, 5):
        LL, s = emit_direction(be, LL, R, S, l, s, OUTT, TOFF[l], PPC, h, g)
        R, S = R // 2, S // 2
    # LL: [64, 4] = 2x2.  scale1 Haar -> 1x1
    CT = be.alloc("ct", [PPC, 2])
    L3 = be.r3(LL, 2)
    be.stt(be.r3(CT, 1), L3[:, :, 0:1], 1.0, L3[:, :, 1:2])
    LL11 = be.alloc("ll11", [PPC, 1])
    be.stt(LL11[:, 0:1], CT[:, 0:1], 1.0, CT[:, 1:2])
    # scales 1-3 for all 6 directions: rank-1 map of LL11 (consts incl. s)
    be.ts_mul(OUTT[:, TCONST:TCONST + 126], TMAP[:, :], LL11[:, 0:1])
    be.store_outt(OUTT)
    return s * c * c  # scale of LL11 (true = s11 * raw); informational


# ---- host-side constants --------------------------------------------------
def _dwt2_np(x, f0, f1):
    def dwt_last(x):
        n = x.shape[-1]
        m = (n + 1) // 2
        xe = np.pad(x, [(0, 0)] * (x.ndim - 1) + [(1, 1)], mode='edge')
        a = xe[..., 1:2 * m + 1:2]
        b = xe[..., 2:2 * m + 2:2]
        return f1 * a + f0 * b, f0 * a - f1 * b

    lo, hi = dwt_last(x)
    lo, hi = np.swapaxes(lo, -1, -2), np.swapaxes(hi, -1, -2)
    ll, lh = dwt_last(lo)
    hl, hh = dwt_last(hi)
    sw = lambda t: np.swapaxes(t, -1, -2)
    return sw(ll), sw(lh), sw(hl), sw(hh)


def build_tail_consts(h, g, s11):
    """126 constants: scales 1-3 outputs as multiples of the raw 1x1 LL."""
    c = INV_SQRT2
    h = np.asarray(h, np.float64)
    g = np.asarray(g, np.float64)
    LL = np.ones((1, 1))
    vals = []
    for k in range(1, 4):
        if k > 1:
            LL, _, _, _ = _dwt2_np(LL, c, c)
        for l in range(6):
            LL, LH, HL, HH = _dwt2_np(LL, h[l, 0], h[l, 1])
            A1, H1, V1, _ = _dwt2_np(LH, g[l, 0], g[l, 1])
            A2, H2, _, _ = _dwt2_np(HL, g[l, 0], g[l, 1])
            _, H3, _, D3 = _dwt2_np(HH, g[l, 0], g[l, 1])
            for sb in (A1, H1, V1, A2, H2, H3, D3):
                vals.append(float(sb[0, 0]))
    return (np.asarray(vals, np.float64) * s11).astype(np.float32)


def tail_scale(h, g):
    """scale s11 of the raw 1x1 LL value (true = s11 * raw)."""
    c = INV_SQRT2
    s = c ** 4  # L1 + L2 Haar drops
    for l in range(6):
        s *= float(h[l, 0]) ** 2
    return s * c * c  # scale1 Haar drops


def build_perm():
    """perm[ref_pos] = index into per-plane packed vector
    v = concat(OUT_BLK rows for blocks 0..7 (8*1176), OUT_TAIL row (273))."""
    perm = np.empty(9681, np.int64)
    off = 0
    for l, m in enumerate((32, 16, 8)):
        rpb = m // NBLK
        loff = LOFF[l]
        for sb in range(7):
            for row in range(m):
                blk, rl = divmod(row, rpb)
                base = blk * BLK_FLOATS + loff + sb * rpb * m + rl * m
                perm[off + sb * m * m + row * m:off + sb * m * m + (row + 1) * m] = \
                    np.arange(base, base + m)
        off += 7 * m * m
    tail_base = NBLK * BLK_FLOATS
    for l, m in ((3, 4), (4, 2), (5, 1)):
        n = 7 * m * m
        perm[off:off + n] = tail_base + TOFF[l] + np.arange(n)
        off += n
    perm[off:off + 126] = tail_base + TCONST + np.arange(126)
    assert off + 126 == 9681
    return perm


def gather_host(out_blk, out_tail, perm):
    """[512,1176],[64,273] per core -> [64, 9681] in reference order."""
    v = np.concatenate(
        [out_blk.astype(np.float32).reshape(
            NBATCH, BPL, NBLK * BLK_FLOATS).reshape(PPC, -1),
         np.asarray(out_tail, np.float32)], axis=1)
    return v[:, perm]


# ---- device backend -------------------------------------------------------
class BassBE:
    """Emits the op plan as a Tile program."""

    def __init__(self, tc, pools, xs_ap, tmap_ap, outblk_ap, outtail_ap,
                 dram_bounce, w4_ap=None, ll2d_ap=None, opts=None):
        self.opts = opts or {}
        self.tc = tc
        self.nc = tc.nc
        self.pools = pools
        self.xs = xs_ap          # [64, 512, 512] dram
        self.tmap_dram = tmap_ap  # [64, 126] dram
        self.outblk = outblk_ap  # [512, 1176] dram
        self.outtail = outtail_ap  # [64, 273] dram
        self.bounce = dram_bounce  # [128, 32] dram scratch
        self.w4_dram = w4_ap     # [128, 32] dram
        self.ll2d = ll2d_ap      # [64, 128, 128] dram scratch

    def alloc(self, name, shape):
        from concourse import mybir
        if name in ('tailll', 'tmap', 'w4t'):
            pool = self.pools['persist']
        elif name == 'll2':
            pool = self.pools['big']
        elif name in ('xt', 'xg'):
            pool = self.pools['xt']
        else:
            pool = self.pools['work']
        dt = (mybir.dt.float32 if name in ('outt', 'll11')
              else mybir.dt.bfloat16)
        return pool.tile(list(shape), dt, tag=name, name=name)

    @staticmethod
    def r3(tile, cols, sub=None):
        ap = tile[:, :] if not hasattr(tile, 'ap') else tile[:, :]
        if sub is not None:
            ap = ap[:, sub[0]:sub[0] + sub[1]]
        P, F = ap.shape
        return ap.rearrange("p (r c) -> p r c", c=cols)

    def stt(self, out, a, s, b):
        from concourse import mybir
        self.nc.vector.scalar_tensor_tensor(
            out=out, in0=a, scalar=float(s), in1=b,
            op0=mybir.AluOpType.mult, op1=mybir.AluOpType.add)

    def tt(self, out, a, b):
        self.nc.vector.tensor_add(out, a, b)

    def copy(self, out, inp):
        self.nc.scalar.copy(out, inp)

    def scale_copy(self, out, inp, s):
        if self.opts.get('comp_engine', 'scalar') == 'vector':
            from concourse import mybir
            self.nc.vector.tensor_scalar(
                out=out, in0=inp, scalar1=float(s), scalar2=None,
                op0=mybir.AluOpType.mult)
        else:
            self.nc.scalar.mul(out, inp, float(s))

    def ts_mul(self, out, a, col):
        from concourse import mybir
        self.nc.vector.tensor_scalar(
            out=out, in0=a, scalar1=col, scalar2=None,
            op0=mybir.AluOpType.mult)

    def load_x_chunk(self, t, sc, dst, nsc=NSC):
        v = self.xs.rearrange("pl (blk s r) c -> pl blk s r c", blk=NBLK, s=nsc)
        v = v[t * BPL:(t + 1) * BPL, :, sc]
        v = v.rearrange("pl blk r c -> (pl blk) (r c)")
        self.nc.sync.dma_start(out=dst[:, :], in_=v)

    # ---- plan C: TensorE phase A -----------------------------------------
    def alloc_psum(self, name, shape):
        from concourse import mybir
        return self.pools['psum'].tile(list(shape), mybir.dt.float32,
                                       tag=name, name=name)

    def load_w4(self, dst):
        self.nc.sync.dma_start(out=dst[:, :], in_=self.w4_dram)

    def load_x_group(self, grp, dst):
        v = self.xs.rearrange("pl (p r) c -> p pl r c", r=4)
        v = v[:, 4 * grp:4 * grp + 4]
        d4 = dst[:, :].rearrange("p (pl r c) -> p pl r c", pl=4, r=4)
        self.nc.sync.dma_start(out=d4, in_=v)

    def mm_rowpair2(self, PS, W4, XT, pl, r4):
        x3 = XT[:, :].rearrange("p (pl r c) -> p pl r c", pl=4, r=4)
        out = PS[:, :].rearrange("p (pl c) -> p pl c", pl=4)
        self.nc.tensor.matmul(out[:, pl], W4[:, :], x3[:, pl, r4],
                              start=(r4 == 0), stop=(r4 == 3))

    def store_ll2_group(self, grp, C2):
        dst = self.ll2d[4 * grp:4 * grp + 4].rearrange("pl r c -> r pl c")
        src = C2[:, :].rearrange("p (pl c) -> p pl c", pl=4)
        self.nc.sync.dma_start(out=dst, in_=src)

    def load_ll2_batch(self, t, dst):
        src = self.ll2d[BPL * t:BPL * (t + 1)].rearrange(
            "pl (blk j) c -> (pl blk) (j c)", blk=NBLK)
        self.nc.sync.dma_start(out=dst[:, :], in_=src)

    def repack_tail(self, t, ll, tail):
        # [128, 32] sbuf -> dram bounce -> tail[16t:16t+16, :] ([16, 256])
        self.nc.sync.dma_start(out=self.bounce[:, :], in_=ll[:, :])
        src = self.bounce.rearrange("(pl b) j -> pl (b j)", b=NBLK)
        self.nc.sync.dma_start(out=tail[t * BPL:(t + 1) * BPL, :], in_=src)

    def store_outb(self, t, outb):
        self.nc.sync.dma_start(
            out=self.outblk[t * 128:(t + 1) * 128, :], in_=outb[:, :])

    def store_outt(self, outt):
        self.nc.sync.dma_start(out=self.outtail[:, :], in_=outt[:, :])

    def load_tmap(self, dst):
        self.nc.sync.dma_start(out=dst[:, :], in_=self.tmap_dram[:, :])


def build_program(h, g, opts=None):
    """Builds the single-core SPMD Tile program. Returns compiled nc."""
    from contextlib import ExitStack
    import concourse.bacc as bacc
    import concourse.tile as tile
    from concourse import mybir

    opts = opts or {}
    nc = bacc.Bacc("TRN2", target_bir_lowering=False, debug=False,
                   num_devices=NCORES)
    xs = nc.dram_tensor("xs", [PPC, 512, 512], mybir.dt.bfloat16,
                        kind="ExternalInput").ap()
    tmap = nc.dram_tensor("tmap", [PPC, 126], mybir.dt.bfloat16,
                          kind="ExternalInput").ap()
    outblk = nc.dram_tensor("out_blk", [NBATCH * 128, BLK_FLOATS],
                            mybir.dt.bfloat16, kind="ExternalOutput").ap()
    outtail = nc.dram_tensor("out_tail", [PPC, TAIL_FLOATS],
                             mybir.dt.float32, kind="ExternalOutput").ap()
    bounce = nc.dram_tensor("bounce", [128, 32], mybir.dt.bfloat16).ap()
    w4 = nc.dram_tensor("w4", [128, 128], mybir.dt.bfloat16,
                        kind="ExternalInput").ap()
    ll2d = nc.dram_tensor("ll2d", [PPC, 128, 128], mybir.dt.bfloat16).ap()

    with ExitStack() as ctx:
        tc = ctx.enter_context(tile.TileContext(nc, trace_sim=False))
        pools = {
            'work': ctx.enter_context(
                tc.tile_pool(name="work", bufs=opts.get('work_bufs', 2))),
            'xt': ctx.enter_context(
                tc.tile_pool(name="xt", bufs=opts.get('xt_bufs', 2))),
            'big': ctx.enter_context(
                tc.tile_pool(name="big", bufs=opts.get('big_bufs', 4))),
            'chain': ctx.enter_context(
                tc.tile_pool(name="chain", bufs=opts.get('chain_bufs', 8))),
            'persist': ctx.enter_context(tc.tile_pool(name="persist", bufs=1)),
            'psum': ctx.enter_context(
                tc.tile_pool(name="psum", bufs=2, space="PSUM")),
        }
        be = BassBE(tc, pools, xs, tmap, outblk, outtail, bounce,
                    w4_ap=w4, ll2d_ap=ll2d, opts=opts)
        for _ in range(opts.get('repeat', 1)):
            emit_core(be, h, g)
    nc.compile()
    return nc


# ---- public entry ---------------------------------------------------------
_CACHE = {}


def kernel(x, h, g):
    import ml_dtypes
    x = np.asarray(x)
    h = np.asarray(h, np.float32)
    g = np.asarray(g, np.float32)
    B, C = x.shape[0], x.shape[1]

    from concourse.bass_utils import run_bass_kernel_spmd

    key = (h.tobytes(), g.tobytes())
    if key not in _CACHE:
        nc = build_program(h, g, {'xt_bufs': 3, 'work_bufs': 2, 'big_bufs': 1, 'c_batches': (), 'nsc': 4})
        tmap_row = build_tail_consts(h, g, tail_scale(h, g))
        tmap = np.ascontiguousarray(
            np.broadcast_to(tmap_row, (PPC, 126))).astype(ml_dtypes.bfloat16)
        perm = build_perm()
        _CACHE[key] = (nc, tmap, perm)
    nc, tmap, perm = _CACHE[key]

    planes = np.ascontiguousarray(x.astype(ml_dtypes.bfloat16)
                                  ).reshape(NPLANES, 512, 512)
    w4 = np.ascontiguousarray(w4_matrix().astype(ml_dtypes.bfloat16))
    in_maps = [{"xs": planes[k * PPC:(k + 1) * PPC], "tmap": tmap, "w4": w4}
               for k in range(NCORES)]
    res = run_bass_kernel_spmd(nc, in_maps, list(range(NCORES)))
    global LAST_EXEC_NS
    LAST_EXEC_NS = getattr(res, 'exec_time_ns', None)
    out = np.empty((NPLANES, 9681), np.float32)
    for k in range(NCORES):
        out[k * PPC:(k + 1) * PPC] = gather_host(
            res.results[k]["out_blk"], res.results[k]["out_tail"], perm)
    return out.reshape(B, C, 9681)

